# revision 18
# baseline (speedup 1.0000x reference)
"""Trainium2 Bass kernel for nn_Autoencoder (point-cloud GNN autoencoder).

Data-parallel over batch: 8 point clouds -> 8 NeuronCores. Per core: kNN via
bf16 hi/lo-split distance matmul + OR-index-packed top-k scan on DVE, then 4
graph-conv layers with AdaIN. Neighbor features fetched with bulk dma_gather
(mlp Q7 library, wrapped-int16 indices built via a DRAM bounce + replication
matmul); all dense matmuls in bf16.
"""
import sys, types, ctypes, contextlib
sys.path.insert(0, '/opt/trn_rl_repo')

import numpy as np
import ml_dtypes
import bass_rust
from concourse import bass, mybir, bass_isa
from concourse import library_config
from concourse.tile import TileContext

B, V, NB, SUP = 8, 2048, 20, 4
NT = V // 128          # 16 point tiles per core
GC = 640              # idxs per gather chunk: 4 equal chunks on 4 queues
F32 = mybir.dt.float32
BF16 = mybir.dt.bfloat16
I32 = mybir.dt.int32
I16 = mybir.dt.int16
AF = mybir.ActivationFunctionType
ALU = mybir.AluOpType


def _split_excess_waits(nc, max_waits=1):
    """Walrus here rejects >1 sync waits per instruction; move extras onto
    NOPs on the same engine right before it."""
    for f in nc.m.functions:
        for bb in f.blocks:
            insts = list(bb.instructions)
            out = []
            for inst in insts:
                si = getattr(inst, 'sync_info', None)
                if si is not None and si.on_wait and len(si.on_wait) > max_waits:
                    waits = list(si.on_wait)
                    move, keep = waits[:-max_waits], waits[-max_waits:]
                    for w in move:
                        eng = nc.engines[inst.engine]
                        nop = eng.nop(nofuse=True)
                        ni = nop.ins
                        for f2 in nc.m.functions:
                            for bb2 in f2.blocks:
                                if ni in bb2.instructions:
                                    bb2.instructions.remove(ni)
                        ni.sync_info = bass_rust.SyncInfo(on_wait=[w], on_update=[])
                        out.append(ni)
                    si.on_wait = keep
                out.append(inst)
            bb.instructions[:] = out


def _encode_reloads(nc):
    """codegen InstPseudoReloadLibraryIndex into raw ISA bytes (walrus can't)."""
    for f in nc.m.functions:
        for bb in f.blocks:
            for pos, inst in enumerate(list(bb.instructions)):
                if isinstance(inst, bass_isa.InstPseudoReloadLibraryIndex):
                    lowered = mybir.codegen_inst_isa_one(inst, nc._state, nc.isa)
                    if not isinstance(lowered, list):
                        lowered = [lowered]
                    bb.instructions[pos:pos + 1] = list(lowered)


def _dma_gather_raw(g, out_ap, in_ap, idxs_ap, num_idxs, num_idxs_reg, elem_size,
                    queue_num=0):
    """InstDMAGatherAnt without the 256B-elem restriction (HW-validated to 64B)."""
    from concourse._compat import exact_div
    stride_bytes = in_ap.ap[0][0] * mybir.dt.size(in_ap.dtype)
    return g.add_instruction(
        mybir.InstDMAGatherAnt(
            name=g.bass.get_next_instruction_name(),
            ins=[*g.lower_ap_dma(in_ap, for_custom_bir_dma=True),
                 g.lower_ap(idxs_ap), g.lower_val_access(num_idxs_reg)],
            outs=[g.lower_ap(out_ap)],
            transpose=False, num_idxs=num_idxs, elem_size=elem_size,
            stride_bytes_256=exact_div(stride_bytes, 256), gen_mode=0,
            single_packet=True, queue_num=queue_num,
            sbuf_tokens_per_rank=0, sbuf_free_dim_per_rank=0,
            sbuf_free_dim_pad_per_rank=0, sbuf_byte_offset=0,
        ))


def _normalize_cols(d):
    n = np.sqrt((d.astype(np.float32) ** 2).sum(0))
    return (d / np.maximum(n, 1e-12)).astype(np.float32)


def _block_dirs(dirsn, K):
    """(3, K) normalized dirs -> block-diagonal (60, NB*K): row (r,d), col (r,k)."""
    bd = np.zeros((3 * NB, NB * K), np.float32)
    for r in range(NB):
        bd[3 * r:3 * r + 3, K * r:K * (r + 1)] = dirsn
    return bd


def build_kernel():
    nc = bass.Bass(num_swdge_queues=4)
    src = nc.dram_tensor("source", [V, 3], F32, kind="ExternalInput")
    tf = nc.dram_tensor("target_feature", [V, 10], F32, kind="ExternalInput")
    # host-packed constants (bf16 for matmul operands)
    bd0 = nc.dram_tensor("bd0", [60, NB * 64], BF16, kind="ExternalInput")
    bd1 = nc.dram_tensor("bd1", [60, NB * 128], BF16, kind="ExternalInput")
    bd2 = nc.dram_tensor("bd2", [60, NB * 64], BF16, kind="ExternalInput")
    bd3 = nc.dram_tensor("bd3", [60, NB * 12], BF16, kind="ExternalInput")
    wb1 = nc.dram_tensor("wb1", [17, 160], BF16, kind="ExternalInput")
    wba = nc.dram_tensor("wba", [11, 64], BF16, kind="ExternalInput")
    wbd1 = nc.dram_tensor("wbd1", [33, 80], BF16, kind="ExternalInput")
    wbd2 = nc.dram_tensor("wbd2", [17, 15], BF16, kind="ExternalInput")
    repsel = nc.dram_tensor("repsel", [16, 128], F32, kind="ExternalInput")
    lhs13i = nc.dram_tensor("lhs13i", [13, V], BF16, kind="ExternalInput")
    rhs13i = nc.dram_tensor("rhs13i", [13, V], BF16, kind="ExternalInput")
    identin = nc.dram_tensor("identin", [128, 128], F32, kind="ExternalInput")
    iota32 = nc.dram_tensor("iota32", [128, V], I32, kind="ExternalInput")
    out = nc.dram_tensor("out", [V, 3], F32, kind="ExternalOutput")
    # DRAM scratch: idx bounce + feature tables (rows = points)
    d1 = nc.dram_tensor("d1", [V, 20], F32)
    tv = nc.dram_tensor("tv", [V, 64], F32)      # padded verts (only cols 0:3)
    t1 = nc.dram_tensor("t1", [V, 128], BF16)    # conv1 supp (256B rows)
    t2 = nc.dram_tensor("t2", [V, 128], BF16)   # dc1 supp bf16 (256B pitch, 64 used)
    t3 = nc.dram_tensor("t3", [V, 128], BF16)    # dc2 supp bf16 (256B pitch, 12 used)

    NGB = GC // 128            # gather blocks (ranks) per dma_gather chunk
    NCH = (NB + NGB - 1) // NGB  # chunks per tile-layer gather (20/NGB)
    WCOLS = GC // 16           # W columns per chunk

    with TileContext(nc) as tc:
        with (
            tc.tile_pool(name="big", bufs=3) as big,
            tc.tile_pool(name="mid", bufs=3) as mid,
            tc.tile_pool(name="sml", bufs=4) as sml,
            tc.tile_pool(name="keep", bufs=1) as keep,
            tc.tile_pool(name="ps", bufs=2, space="PSUM") as ps,
            tc.tile_pool(name="ps2", bufs=1, space="PSUM") as ps2,
        ):
            nc.gpsimd.load_library(library_config.mlp)
            nreg = {n: nc.gpsimd.to_reg(n) for n in (512, 640, 1024)}
            tv16 = bass.AP(tv[:].tensor, 0, [[64, V], [1, 16]])
            t1v = None  # set below

            ident = keep.tile([128, 128], F32)
            nc.sync.dma_start(out=ident[:], in_=identin[:])
            iot = keep.tile([128, V], I32)
            nc.sync.dma_start(out=iot[:], in_=iota32[:])
            rsel = keep.tile([16, 128], F32)
            nc.sync.dma_start(out=rsel[:], in_=repsel[:])

            lhsT13 = keep.tile([13, V], BF16)
            nc.sync.dma_start(out=lhsT13[:], in_=lhs13i[:])
            rhs13 = keep.tile([13, V], BF16)
            nc.sync.dma_start(out=rhs13[:], in_=rhs13i[:])

            # padded verts table in DRAM
            nc.sync.dma_start(
                out=bass.AP(tv[:].tensor, 0, [[64, V], [1, 3]]),
                in_=bass.AP(src[:].tensor, 0, [[3, V], [1, 3]]))

            # ---- weight constants ----
            wb1s = keep.tile([17, 160], BF16)
            nc.sync.dma_start(out=wb1s[:], in_=wb1[:])
            wbas = keep.tile([11, 64], BF16)
            nc.sync.dma_start(out=wbas[:], in_=wba[:])
            wbd1s = keep.tile([33, 80], BF16)
            nc.sync.dma_start(out=wbd1s[:], in_=wbd1[:])
            wbd2s = keep.tile([17, 15], BF16)
            nc.sync.dma_start(out=wbd2s[:], in_=wbd2[:])
            bd0s = keep.tile([60, NB * 64], BF16)
            nc.sync.dma_start(out=bd0s[:], in_=bd0[:])
            bd1s = keep.tile([60, NB * 128], BF16)
            nc.sync.dma_start(out=bd1s[:], in_=bd1[:])
            bd2s = keep.tile([60, NB * 64], BF16)
            nc.sync.dma_start(out=bd2s[:], in_=bd2[:])
            bd3s = keep.tile([60, NB * 12], BF16)
            nc.sync.dma_start(out=bd3s[:], in_=bd3[:])

            # ---- persistent state ----
            W_all = keep.tile([128, NT * 160], I16)   # wrapped gather idxs
            dnT_all = keep.tile([60, V], BF16)
            vts = keep.tile([128, NT * 3], F32)
            f1_all = keep.tile([128, NT * 16], F32)
            f2_all = keep.tile([128, NT * 32], F32)
            t_all = keep.tile([128, NT * 32], F32)
            c1_all = keep.tile([128, NT * 16], F32)
            cen1 = keep.tile([128, NT * 32], F32)
            cend1 = keep.tile([128, NT * 16], F32)
            cend2 = keep.tile([128, NT * 3], F32)
            s1acc = keep.tile([1, 32], F32)
            s2acc = keep.tile([1, 32], F32)
            nc.vector.memset(s1acc[:], 0.0)
            nc.vector.memset(s2acc[:], 0.0)
            ones128 = keep.tile([128, 1], F32)
            nc.vector.memset(ones128[:], 1.0)

            # persistent feat lhsT tiles with ones rows preset
            onesrow = keep.tile([1, 128], BF16)
            nc.vector.memset(onesrow[:], 1.0)
            lt1 = keep.tile([17, 128], BF16)
            nc.sync.dma_start(out=lt1[16:17, :], in_=onesrow[:])
            lta = keep.tile([11, 128], BF16)
            nc.sync.dma_start(out=lta[10:11, :], in_=onesrow[:])
            ltd1 = keep.tile([33, 128], BF16)
            nc.sync.dma_start(out=ltd1[32:33, :], in_=onesrow[:])
            ltd2 = keep.tile([17, 128], BF16)
            nc.sync.dma_start(out=ltd2[16:17, :], in_=onesrow[:])

            def gather(t, tview, C, dest, gc=GC):
                """dest (128, NB*C) <- table-view rows per W_all chunk idxs."""
                ngb = gc // 128
                nch = (NB + ngb - 1) // ngb
                for ch in range(nch):
                    nblk = min(ngb, NB - ch * ngb)
                    n = nblk * 128
                    dv = bass.AP(dest.tensor,
                                 dest[:].offset + ch * ngb * C,
                                 [[dest[:].ap[0][0], 128], [C, nblk], [1, C]])
                    col0 = t * 160 + ch * (gc // 16)
                    _dma_gather_raw(
                        nc.gpsimd, dv, tview,
                        W_all[:, col0: col0 + (n // 16)],
                        n, nreg[n], C, queue_num=ch % 4)

            def theta_relu(t, bds, K, dest):
                """dest (128, NB*K) bf16 = relu(dnT_t.T @ block dirs)."""
                n = NB * K
                dT = dnT_all[:, t * 128:(t + 1) * 128]
                for j in range(0, n, 512):
                    w = min(512, n - j)
                    tp = ps.tile([128, 512], F32, tag="theta")
                    nc.tensor.matmul(out=tp[:, :w], lhsT=dT,
                                     rhs=bds[:, j:j + w], start=True, stop=True)
                    nc.scalar.activation(out=dest[:, j:j + w], in_=tp[:, :w], func=AF.Relu)

            def feat_matmul(t, fmap_ap, cin, lt, wbs, nout):
                """feat psum (128, nout) = [fmap | 1] @ [w; b] for tile t."""
                ftp = ps2.tile([cin, 128], F32, tag="ftp")
                nc.tensor.transpose(out=ftp[:], in_=fmap_ap, identity=ident[:])
                nc.scalar.copy(out=lt[:cin, :], in_=ftp[:])
                fp = ps2.tile([128, nout], F32, tag="feat")
                nc.tensor.matmul(out=fp[:], lhsT=lt[:], rhs=wbs[:], start=True, stop=True)
                return fp

            # ================= pass 0: dist + topk + idx + dn + conv0 ============
            for t in range(NT):
                scr = big.tile([128, V], F32, tag="scr")
                for j in range(4):
                    nd_ps = ps.tile([128, 512], F32, tag="nd")
                    nc.tensor.matmul(out=nd_ps[:],
                                     lhsT=lhsT13[:, bass.ts(t, 128)],
                                     rhs=rhs13[:, bass.ts(j, 512)], start=True, stop=True)
                    nc.vector.tensor_scalar(
                        out=scr[:, bass.ts(j, 512)].bitcast(I32),
                        in0=nd_ps[:].bitcast(I32), scalar1=-2048,
                        scalar2=None, op0=ALU.bitwise_and)
                    nc.vector.tensor_tensor(
                        out=scr[:, bass.ts(j, 512)].bitcast(I32),
                        in0=scr[:, bass.ts(j, 512)].bitcast(I32),
                        in1=iot[:, bass.ts(j, 512)], op=ALU.bitwise_or)
                v24 = sml.tile([128, 24], F32, tag="v24")
                nc.vector.max(out=v24[:, 0:8], in_=scr[:])
                nc.vector.match_replace(out=scr[:], in_to_replace=v24[:, 0:8],
                                        in_values=scr[:], imm_value=-3.0e38)
                nc.vector.max(out=v24[:, 8:16], in_=scr[:])
                nc.vector.match_replace(out=scr[:], in_to_replace=v24[:, 8:16],
                                        in_values=scr[:], imm_value=-3.0e38)
                nc.vector.max(out=v24[:, 16:24], in_=scr[:])
                ki = sml.tile([128, 24], I32, tag="ki")
                nc.vector.tensor_scalar(out=ki[:], in0=v24[:].bitcast(I32),
                                        scalar1=0x7FF, scalar2=None, op0=ALU.bitwise_and)
                kf = sml.tile([128, 24], F32, tag="kf")
                nc.vector.tensor_copy(out=kf[:], in_=ki[:])
                # idx bounce -> wrapped int16 W
                nc.sync.dma_start(out=d1[t * 128:(t + 1) * 128, :], in_=kf[:, 1:21])
                ib = sml.tile([16, 160], F32, tag="ib")
                nc.sync.dma_start(
                    out=bass.AP(ib.tensor, ib[:].offset,
                                [[ib[:].ap[0][0], 16], [20, 8], [1, 20]]),
                    in_=bass.AP(d1[:].tensor, t * 128 * 20,
                                [[20, 16], [320, 8], [1, 20]]))
                jb = sml.tile([16, 160], F32, tag="jb")
                nc.vector.tensor_copy(
                    out=bass.AP(jb.tensor, jb[:].offset,
                                [[jb[:].ap[0][0], 16], [8, 20], [1, 8]]),
                    in_=bass.AP(ib.tensor, ib[:].offset,
                                [[ib[:].ap[0][0], 16], [1, 20], [20, 8]]))
                wp = ps2.tile([128, 160], F32, tag="wp")
                nc.tensor.matmul(out=wp[:], lhsT=rsel[:], rhs=jb[:], start=True, stop=True)
                nc.scalar.copy(out=W_all[:, t * 160:(t + 1) * 160], in_=wp[:])

                # verts of this tile + bulk-gathered neighbor verts
                vt = vts[:, t * 3:(t + 1) * 3]
                nc.sync.dma_start(out=vt, in_=src[t * 128:(t + 1) * 128, :])
                vg = mid.tile([128, NB * 16], F32, tag="vg")
                gather(t, tv16, 16, vg, gc=1024)
                vgv = bass.AP(vg.tensor, vg[:].offset,
                              [[vg[:].ap[0][0], 128], [16, NB], [1, 3]])
                dv = mid.tile([128, NB * 3], F32, tag="dv")
                vt_b = bass.AP(vts[:].tensor, vts[:].offset + t * 3,
                               [[NT * 3, 128], [0, NB], [1, 3]])
                nc.vector.tensor_tensor(out=dv[:], in0=vgv, in1=vt_b, op=ALU.subtract)
                dsq = mid.tile([128, NB * 3], F32, tag="dsq")
                nc.vector.tensor_mul(out=dsq[:], in0=dv[:], in1=dv[:])
                nsq = sml.tile([128, NB], F32, tag="nsq")
                nc.vector.tensor_reduce(
                    out=nsq[:], in_=dsq[:].rearrange("p (r d) -> p r d", r=NB, d=3),
                    axis=mybir.AxisListType.X, op=ALU.add)
                rn = sml.tile([128, NB], F32, tag="rn")
                nc.scalar.activation(out=rn[:], in_=nsq[:], func=AF.Sqrt)
                nc.vector.tensor_scalar_max(rn[:], rn[:], 1e-12)
                nc.vector.reciprocal(out=rn[:], in_=rn[:])
                dn = mid.tile([128, NB * 3], F32, tag="dn")
                rn_b = bass.AP(rn.tensor, rn[:].offset,
                               [[rn[:].ap[0][0], 128], [1, NB], [0, 3]])
                nc.vector.tensor_tensor(out=dn[:], in0=dv[:], in1=rn_b, op=ALU.mult)
                dnp = ps2.tile([60, 128], F32, tag="ftp")
                nc.tensor.transpose(out=dnp[:], in_=dn[:, :60], identity=ident[:])
                nc.scalar.copy(out=dnT_all[:, t * 128:(t + 1) * 128], in_=dnp[:])

                # conv0: theta only -> f1
                th0 = mid.tile([128, NB * 64], BF16, tag="th")
                theta_relu(t, bd0s, 64, th0)
                mx = sml.tile([128, 64], F32, tag="mx64")
                nc.vector.tensor_reduce(
                    out=mx[:], in_=bass.AP(th0.tensor, th0[:].offset,
                                           [[th0[:].ap[0][0], 128], [1, 64], [64, NB]]),
                    axis=mybir.AxisListType.X, op=ALU.max)
                f1t = f1_all[:, t * 16:(t + 1) * 16]
                nc.vector.tensor_reduce(
                    out=f1t, in_=bass.AP(mx.tensor, mx[:].offset,
                                         [[mx[:].ap[0][0], 128], [1, 16], [16, 4]]),
                    axis=mybir.AxisListType.X, op=ALU.add)
                nc.vector.tensor_scalar_max(f1t, f1t, 0.0)
                # conv1 feature table + cached center
                fp = feat_matmul(t, f1t, 16, lt1, wb1s[:], 160)
                nc.scalar.copy(out=cen1[:, t * 32:(t + 1) * 32], in_=fp[:, 0:32])
                sup = sml.tile([128, 128], BF16, tag="sup1")
                nc.scalar.copy(out=sup[:], in_=fp[:, 32:160])
                nc.sync.dma_start(out=t1[t * 128:(t + 1) * 128, :], in_=sup[:])

            # ================= pass 1: conv1 -> f2, adain stats =================
            for t in range(NT):
                sg = mid.tile([128, NB * 128], BF16, tag="sg")
                gather(t, t1[:], 128, sg)
                th = mid.tile([128, NB * 128], BF16, tag="th")
                theta_relu(t, bd1s, 128, th)
                nc.vector.tensor_mul(out=th[:], in0=th[:], in1=sg[:])
                mx = sml.tile([128, 128], F32, tag="mx128")
                nc.vector.tensor_reduce(
                    out=mx[:], in_=bass.AP(th.tensor, th[:].offset,
                                           [[th[:].ap[0][0], 128], [1, 128], [128, NB]]),
                    axis=mybir.AxisListType.X, op=ALU.max)
                acc = sml.tile([128, 32], F32, tag="acc32")
                nc.vector.tensor_reduce(
                    out=acc[:], in_=bass.AP(mx.tensor, mx[:].offset,
                                            [[mx[:].ap[0][0], 128], [1, 32], [32, 4]]),
                    axis=mybir.AxisListType.X, op=ALU.add)
                f2t = f2_all[:, t * 32:(t + 1) * 32]
                nc.vector.tensor_add(out=acc[:], in0=acc[:], in1=cen1[:, t * 32:(t + 1) * 32])
                nc.scalar.activation(out=f2t, in_=acc[:], func=AF.Relu)
                # adain stats
                sp = ps2.tile([1, 64], F32, tag="sp")
                nc.tensor.matmul(out=sp[:, 0:32], lhsT=ones128[:], rhs=f2t, start=True, stop=True)
                f2sq = sml.tile([128, 32], F32, tag="f2sq")
                nc.vector.tensor_mul(out=f2sq[:], in0=f2t, in1=f2t)
                nc.tensor.matmul(out=sp[:, 32:64], lhsT=ones128[:], rhs=f2sq[:], start=True, stop=True)
                nc.vector.tensor_add(out=s1acc[:], in0=s1acc[:], in1=sp[:, 0:32])
                nc.vector.tensor_add(out=s2acc[:], in0=s2acc[:], in1=sp[:, 32:64])

            # ---- adain finalize ----
            stat = keep.tile([1, 64], F32)
            nc.vector.tensor_scalar_mul(stat[:, 0:32], s1acc[:], 1.0 / V)
            m2 = keep.tile([1, 32], F32)
            nc.vector.tensor_mul(out=m2[:], in0=stat[:, 0:32], in1=s1acc[:])
            nc.vector.tensor_sub(out=m2[:], in0=s2acc[:], in1=m2[:])
            nc.vector.tensor_scalar_mul(m2[:], m2[:], 1.0 / (V - 1))
            nc.scalar.activation(out=m2[:], in_=m2[:], func=AF.Sqrt)
            nc.vector.tensor_scalar_add(m2[:], m2[:], 1e-8)
            nc.vector.reciprocal(out=stat[:, 32:64], in_=m2[:])
            ones1 = keep.tile([1, 128], F32)
            nc.vector.memset(ones1[:], 1.0)
            bc_ps = ps2.tile([128, 64], F32, tag="feat")
            nc.tensor.matmul(out=bc_ps[:], lhsT=ones1[:], rhs=stat[:], start=True, stop=True)
            bc = keep.tile([128, 64], F32)
            nc.scalar.copy(out=bc[:], in_=bc_ps[:])

            # ---- pass 1b: t = adain(f2), dc1 table ----
            for t in range(NT):
                tft = sml.tile([128, 10], F32, tag="tft")
                nc.sync.dma_start(out=tft[:], in_=tf[t * 128:(t + 1) * 128, :])
                hp = feat_matmul(t, tft[:], 10, lta, wbas[:], 64)
                f2t = f2_all[:, t * 32:(t + 1) * 32]
                xn = sml.tile([128, 32], F32, tag="xn")
                nc.vector.tensor_sub(out=xn[:], in0=f2t, in1=bc[:, 0:32])
                nc.vector.tensor_mul(out=xn[:], in0=xn[:], in1=bc[:, 32:64])
                g1 = sml.tile([128, 32], F32, tag="g1")
                nc.scalar.add(out=g1[:], in_=hp[:, 0:32], add=1.0)
                nc.vector.tensor_mul(out=xn[:], in0=xn[:], in1=g1[:])
                tt = t_all[:, t * 32:(t + 1) * 32]
                nc.vector.tensor_add(out=tt, in0=xn[:], in1=hp[:, 32:64])
                fp = feat_matmul(t, tt, 32, ltd1, wbd1s[:], 80)
                nc.scalar.copy(out=cend1[:, t * 16:(t + 1) * 16], in_=fp[:, 0:16])
                sup = sml.tile([128, 64], BF16, tag="sup2")
                nc.scalar.copy(out=sup[:], in_=fp[:, 16:80])
                nc.sync.dma_start(
                    out=bass.AP(t2[:].tensor, t * 128 * 128, [[128, 128], [1, 64]]),
                    in_=sup[:])

            # ================= pass 2: dc1 -> c1, dc2 table =================
            for t in range(NT):
                sg = mid.tile([128, NB * 64], BF16, tag="sg")
                gather(t, bass.AP(t2[:].tensor, 0, [[128, V], [1, 64]]), 64, sg, gc=1024)
                th = mid.tile([128, NB * 64], BF16, tag="th")
                theta_relu(t, bd2s, 64, th)
                nc.vector.tensor_mul(out=th[:], in0=th[:], in1=sg[:])
                mx = sml.tile([128, 64], F32, tag="mx128")
                nc.vector.tensor_reduce(
                    out=mx[:], in_=bass.AP(th.tensor, th[:].offset,
                                           [[th[:].ap[0][0], 128], [1, 64], [64, NB]]),
                    axis=mybir.AxisListType.X, op=ALU.max)
                acc = sml.tile([128, 16], F32, tag="acc16")
                nc.vector.tensor_reduce(
                    out=acc[:], in_=bass.AP(mx.tensor, mx[:].offset,
                                            [[mx[:].ap[0][0], 128], [1, 16], [16, 4]]),
                    axis=mybir.AxisListType.X, op=ALU.add)
                c1t = c1_all[:, t * 16:(t + 1) * 16]
                nc.vector.tensor_add(out=acc[:], in0=acc[:], in1=cend1[:, t * 16:(t + 1) * 16])
                nc.scalar.activation(out=c1t, in_=acc[:], func=AF.Relu)
                fp2 = feat_matmul(t, c1t, 16, ltd2, wbd2s[:], 15)
                nc.scalar.copy(out=cend2[:, t * 3:(t + 1) * 3], in_=fp2[:, 0:3])
                sup = sml.tile([128, 12], BF16, tag="sup3")
                nc.scalar.copy(out=sup[:], in_=fp2[:, 3:15])
                nc.sync.dma_start(
                    out=bass.AP(t3[:].tensor, t * 128 * 128, [[128, 128], [1, 12]]),
                    in_=sup[:])

            # ================= pass 3: dc2 -> sigmoid -> out =================
            for t in range(NT):
                sg = mid.tile([128, NB * 32], BF16, tag="sg")
                gather(t, bass.AP(t3[:].tensor, 0, [[128, V], [1, 32]]), 32, sg, gc=1024)
                th = mid.tile([128, NB * 12], BF16, tag="th")
                theta_relu(t, bd3s, 12, th)
                sgv = bass.AP(sg.tensor, sg[:].offset,
                              [[sg[:].ap[0][0], 128], [32, NB], [1, 12]])
                nc.vector.tensor_tensor(out=th[:], in0=th[:], in1=sgv, op=ALU.mult)
                mx = sml.tile([128, 12], F32, tag="mx12")
                nc.vector.tensor_reduce(
                    out=mx[:], in_=bass.AP(th.tensor, th[:].offset,
                                           [[th[:].ap[0][0], 128], [1, 12], [12, NB]]),
                    axis=mybir.AxisListType.X, op=ALU.max)
                acc = sml.tile([128, 3], F32, tag="acc3")
                nc.vector.tensor_reduce(
                    out=acc[:], in_=bass.AP(mx.tensor, mx[:].offset,
                                            [[mx[:].ap[0][0], 128], [1, 3], [3, 4]]),
                    axis=mybir.AxisListType.X, op=ALU.add)
                nc.vector.tensor_add(out=acc[:], in0=acc[:], in1=cend2[:, t * 3:(t + 1) * 3])
                sig = sml.tile([128, 3], F32, tag="sig")
                nc.scalar.activation(out=sig[:], in_=acc[:], func=AF.Sigmoid)
                nc.sync.dma_start(out=out[t * 128:(t + 1) * 128, :], in_=sig[:])

    _split_excess_waits(nc)
    _encode_reloads(nc)
    return nc


_NC_CACHE = None


def _host_consts(inputs):
    bf = ml_dtypes.bfloat16
    repsel = np.zeros((16, 128), np.float32)
    for p in range(128):
        repsel[p % 16, p] = 1.0
    return {
        'bd0': _block_dirs(_normalize_cols(np.asarray(inputs['conv0_dirs'])), 64).astype(bf),
        'bd1': _block_dirs(_normalize_cols(np.asarray(inputs['conv1_dirs'])), 128).astype(bf),
        'bd2': _block_dirs(_normalize_cols(np.asarray(inputs['dc1_dirs'])), 64).astype(bf),
        'bd3': _block_dirs(_normalize_cols(np.asarray(inputs['dc2_dirs'])), 12).astype(bf),
        'wb1': np.vstack([np.asarray(inputs['conv1_w']), np.asarray(inputs['conv1_b'])[None]]).astype(bf),
        'wba': np.vstack([np.asarray(inputs['adain_w']), np.asarray(inputs['adain_b'])[None]]).astype(bf),
        'wbd1': np.vstack([np.asarray(inputs['dc1_w']), np.asarray(inputs['dc1_b'])[None]]).astype(bf),
        'wbd2': np.vstack([np.asarray(inputs['dc2_w']), np.asarray(inputs['dc2_b'])[None]]).astype(bf),
        'repsel': repsel,
        'identin': np.eye(128, dtype=np.float32),
        'iota32': np.tile(np.arange(V, dtype=np.int32)[None, :], (128, 1)),
    }


def _dist_operands(x):
    bf = ml_dtypes.bfloat16
    f = np.float32
    sq = (x * x).sum(1)
    xh = x.astype(bf); xl = (x - xh.astype(f)).astype(bf)
    sqh = sq.astype(bf); sql = (sq - sqh.astype(f)).astype(bf)
    c = -sq; ch = c.astype(bf); cl = (c - ch.astype(f)).astype(bf)
    ones = np.ones(V, bf)
    x2h = (2.0 * xh.astype(f)).astype(bf)
    x2l = (2.0 * xl.astype(f)).astype(bf)
    lhsT = np.stack([*xh.T, *xl.T, *xh.T, sqh, sql, ones, ones])
    rhs = np.stack([*x2h.T, *x2h.T, *x2l.T, -ones, -ones, ch, cl])
    return lhsT.astype(bf), rhs.astype(bf)


def kernel(**inputs):
    global _NC_CACHE
    from concourse.bass_utils import run_bass_kernel_spmd

    src = np.ascontiguousarray(np.asarray(inputs['source'], dtype=np.float32))
    tf = np.ascontiguousarray(np.asarray(inputs['target_feature'], dtype=np.float32))
    consts = {k: np.ascontiguousarray(v) for k, v in _host_consts(inputs).items()}
    if _NC_CACHE is None:
        _NC_CACHE = build_kernel()
    nc = _NC_CACHE
    in_maps = []
    for b in range(B):
        l13, r13 = _dist_operands(src[b])
        in_maps.append(dict(consts, source=src[b], target_feature=tf[b],
                            lhs13i=np.ascontiguousarray(l13),
                            rhs13i=np.ascontiguousarray(r13)))
    res = run_bass_kernel_spmd(nc, in_maps, list(range(B)))
    return np.stack([res.results[b]['out'] for b in range(B)]).astype(np.float32)


if __name__ == '__main__':
    inp = dict(np.load('/root/problem/dev/inputs.npz'))
    o = kernel(**inp)
    print(o.shape, o.dtype)


# revision 19
# speedup vs baseline: 1.1692x; 1.1692x over previous
"""Trainium2 Bass kernel for nn_Autoencoder (point-cloud GNN autoencoder).

Data-parallel over batch: 8 point clouds -> 8 NeuronCores. Per core: kNN via
bf16 hi/lo-split distance matmul + OR-index-packed top-k scan on DVE, then 4
graph-conv layers with AdaIN. Neighbor features fetched with bulk dma_gather
(mlp Q7 library, wrapped-int16 indices built via a DRAM bounce + replication
matmul); all dense matmuls in bf16.
"""
import sys, types, ctypes, contextlib
sys.path.insert(0, '/opt/trn_rl_repo')

import numpy as np
import ml_dtypes
import bass_rust
from concourse import bass, mybir, bass_isa
from concourse import library_config
from concourse.tile import TileContext

B, V, NB, SUP = 8, 2048, 20, 4
NT = V // 128          # 16 point tiles per core
GC = 640              # idxs per gather chunk: 4 equal chunks on 4 queues
F32 = mybir.dt.float32
BF16 = mybir.dt.bfloat16
I32 = mybir.dt.int32
I16 = mybir.dt.int16
AF = mybir.ActivationFunctionType
ALU = mybir.AluOpType


def _split_excess_waits(nc, max_waits=1):
    """Walrus here rejects >1 sync waits per instruction; move extras onto
    NOPs on the same engine right before it."""
    for f in nc.m.functions:
        for bb in f.blocks:
            insts = list(bb.instructions)
            out = []
            for inst in insts:
                si = getattr(inst, 'sync_info', None)
                if si is not None and si.on_wait and len(si.on_wait) > max_waits:
                    waits = list(si.on_wait)
                    move, keep = waits[:-max_waits], waits[-max_waits:]
                    for w in move:
                        eng = nc.engines[inst.engine]
                        nop = eng.nop(nofuse=True)
                        ni = nop.ins
                        for f2 in nc.m.functions:
                            for bb2 in f2.blocks:
                                if ni in bb2.instructions:
                                    bb2.instructions.remove(ni)
                        ni.sync_info = bass_rust.SyncInfo(on_wait=[w], on_update=[])
                        out.append(ni)
                    si.on_wait = keep
                out.append(inst)
            bb.instructions[:] = out


def _encode_reloads(nc):
    """codegen InstPseudoReloadLibraryIndex into raw ISA bytes (walrus can't)."""
    for f in nc.m.functions:
        for bb in f.blocks:
            for pos, inst in enumerate(list(bb.instructions)):
                if isinstance(inst, bass_isa.InstPseudoReloadLibraryIndex):
                    lowered = mybir.codegen_inst_isa_one(inst, nc._state, nc.isa)
                    if not isinstance(lowered, list):
                        lowered = [lowered]
                    bb.instructions[pos:pos + 1] = list(lowered)


def _dma_gather_raw(g, out_ap, in_ap, idxs_ap, num_idxs, num_idxs_reg, elem_size,
                    queue_num=0):
    """InstDMAGatherAnt without the 256B-elem restriction (HW-validated to 64B)."""
    from concourse._compat import exact_div
    stride_bytes = in_ap.ap[0][0] * mybir.dt.size(in_ap.dtype)
    return g.add_instruction(
        mybir.InstDMAGatherAnt(
            name=g.bass.get_next_instruction_name(),
            ins=[*g.lower_ap_dma(in_ap, for_custom_bir_dma=True),
                 g.lower_ap(idxs_ap), g.lower_val_access(num_idxs_reg)],
            outs=[g.lower_ap(out_ap)],
            transpose=False, num_idxs=num_idxs, elem_size=elem_size,
            stride_bytes_256=exact_div(stride_bytes, 256), gen_mode=0,
            single_packet=True, queue_num=queue_num,
            sbuf_tokens_per_rank=0, sbuf_free_dim_per_rank=0,
            sbuf_free_dim_pad_per_rank=0, sbuf_byte_offset=0,
        ))


def _normalize_cols(d):
    n = np.sqrt((d.astype(np.float32) ** 2).sum(0))
    return (d / np.maximum(n, 1e-12)).astype(np.float32)


def _block_dirs(dirsn, K):
    """(3, K) normalized dirs -> block-diagonal (60, NB*K): row (r,d), col (r,k)."""
    bd = np.zeros((3 * NB, NB * K), np.float32)
    for r in range(NB):
        bd[3 * r:3 * r + 3, K * r:K * (r + 1)] = dirsn
    return bd


def build_kernel():
    nc = bass.Bass(num_swdge_queues=4)
    src = nc.dram_tensor("source", [V, 3], F32, kind="ExternalInput")
    tf = nc.dram_tensor("target_feature", [V, 10], F32, kind="ExternalInput")
    # host-packed constants (bf16 for matmul operands)
    bd0 = nc.dram_tensor("bd0", [60, NB * 64], BF16, kind="ExternalInput")
    bd1 = nc.dram_tensor("bd1", [60, NB * 128], BF16, kind="ExternalInput")
    bd2 = nc.dram_tensor("bd2", [60, NB * 64], BF16, kind="ExternalInput")
    bd3 = nc.dram_tensor("bd3", [60, NB * 12], BF16, kind="ExternalInput")
    wb1 = nc.dram_tensor("wb1", [17, 160], BF16, kind="ExternalInput")
    wba = nc.dram_tensor("wba", [11, 64], BF16, kind="ExternalInput")
    wbd1 = nc.dram_tensor("wbd1", [33, 80], BF16, kind="ExternalInput")
    wbd2 = nc.dram_tensor("wbd2", [17, 15], BF16, kind="ExternalInput")
    repsel = nc.dram_tensor("repsel", [16, 128], F32, kind="ExternalInput")
    lhs13i = nc.dram_tensor("lhs13i", [13, V], BF16, kind="ExternalInput")
    rhs13i = nc.dram_tensor("rhs13i", [13, V], BF16, kind="ExternalInput")
    identin = nc.dram_tensor("identin", [128, 128], F32, kind="ExternalInput")
    iota32 = nc.dram_tensor("iota32", [128, V], I32, kind="ExternalInput")
    out = nc.dram_tensor("out", [V, 3], F32, kind="ExternalOutput")
    # DRAM scratch: idx bounce + feature tables (rows = points)
    d1 = nc.dram_tensor("d1", [V, 20], F32)
    tv = nc.dram_tensor("tv", [V, 64], F32)      # padded verts (only cols 0:3)
    t1 = nc.dram_tensor("t1", [V, 128], BF16)    # conv1 supp (256B rows)
    t2 = nc.dram_tensor("t2", [V, 128], BF16)   # dc1 supp bf16 (256B pitch, 64 used)
    t3 = nc.dram_tensor("t3", [V, 128], BF16)    # dc2 supp bf16 (256B pitch, 12 used)

    NGB = GC // 128            # gather blocks (ranks) per dma_gather chunk
    NCH = (NB + NGB - 1) // NGB  # chunks per tile-layer gather (20/NGB)
    WCOLS = GC // 16           # W columns per chunk

    with TileContext(nc) as tc:
        with (
            tc.tile_pool(name="big", bufs=3) as big,
            tc.tile_pool(name="mid", bufs=3) as mid,
            tc.tile_pool(name="sml", bufs=4) as sml,
            tc.tile_pool(name="keep", bufs=1) as keep,
            tc.tile_pool(name="ps", bufs=2, space="PSUM") as ps,
            tc.tile_pool(name="ps2", bufs=1, space="PSUM") as ps2,
        ):
            nc.gpsimd.load_library(library_config.mlp)
            nreg = {n: nc.gpsimd.to_reg(n) for n in
                    sorted({min(NGB, NB - ch * NGB) * 128 for ch in range(NCH)})}
            tv16 = bass.AP(tv[:].tensor, 0, [[64, V], [1, 16]])
            t1v = None  # set below

            ident = keep.tile([128, 128], F32)
            nc.sync.dma_start(out=ident[:], in_=identin[:])
            iot = keep.tile([128, V], I32)
            nc.sync.dma_start(out=iot[:], in_=iota32[:])
            rsel = keep.tile([16, 128], F32)
            nc.sync.dma_start(out=rsel[:], in_=repsel[:])

            lhsT13 = keep.tile([13, V], BF16)
            nc.sync.dma_start(out=lhsT13[:], in_=lhs13i[:])
            rhs13 = keep.tile([13, V], BF16)
            nc.sync.dma_start(out=rhs13[:], in_=rhs13i[:])

            # padded verts table in DRAM
            nc.sync.dma_start(
                out=bass.AP(tv[:].tensor, 0, [[64, V], [1, 3]]),
                in_=bass.AP(src[:].tensor, 0, [[3, V], [1, 3]]))

            # ---- weight constants ----
            wb1s = keep.tile([17, 160], BF16)
            nc.sync.dma_start(out=wb1s[:], in_=wb1[:])
            wbas = keep.tile([11, 64], BF16)
            nc.sync.dma_start(out=wbas[:], in_=wba[:])
            wbd1s = keep.tile([33, 80], BF16)
            nc.sync.dma_start(out=wbd1s[:], in_=wbd1[:])
            wbd2s = keep.tile([17, 15], BF16)
            nc.sync.dma_start(out=wbd2s[:], in_=wbd2[:])
            bd0s = keep.tile([60, NB * 64], BF16)
            nc.sync.dma_start(out=bd0s[:], in_=bd0[:])
            bd1s = keep.tile([60, NB * 128], BF16)
            nc.sync.dma_start(out=bd1s[:], in_=bd1[:])
            bd2s = keep.tile([60, NB * 64], BF16)
            nc.sync.dma_start(out=bd2s[:], in_=bd2[:])
            bd3s = keep.tile([60, NB * 12], BF16)
            nc.sync.dma_start(out=bd3s[:], in_=bd3[:])

            # ---- persistent state ----
            W_all = keep.tile([128, NT * 160], I16)   # wrapped gather idxs
            dnT_all = keep.tile([60, V], BF16)
            vts = keep.tile([128, NT * 3], F32)
            f1_all = keep.tile([128, NT * 16], F32)
            f2_all = keep.tile([128, NT * 32], F32)
            t_all = keep.tile([128, NT * 32], F32)
            c1_all = keep.tile([128, NT * 16], F32)
            cen1 = keep.tile([128, NT * 32], F32)
            cend1 = keep.tile([128, NT * 16], F32)
            cend2 = keep.tile([128, NT * 3], F32)
            s1acc = keep.tile([1, 32], F32)
            s2acc = keep.tile([1, 32], F32)
            nc.vector.memset(s1acc[:], 0.0)
            nc.vector.memset(s2acc[:], 0.0)
            ones128 = keep.tile([128, 1], F32)
            nc.vector.memset(ones128[:], 1.0)

            # persistent feat lhsT tiles with ones rows preset
            onesrow = keep.tile([1, 128], BF16)
            nc.vector.memset(onesrow[:], 1.0)
            lt1 = keep.tile([17, 128], BF16)
            nc.sync.dma_start(out=lt1[16:17, :], in_=onesrow[:])
            lta = keep.tile([11, 128], BF16)
            nc.sync.dma_start(out=lta[10:11, :], in_=onesrow[:])
            ltd1 = keep.tile([33, 128], BF16)
            nc.sync.dma_start(out=ltd1[32:33, :], in_=onesrow[:])
            ltd2 = keep.tile([17, 128], BF16)
            nc.sync.dma_start(out=ltd2[16:17, :], in_=onesrow[:])

            def gather(t, tview, C, dest):
                """dest (128, NB*C) <- table-view rows per W_all chunk idxs."""
                for ch in range(NCH):
                    nblk = min(NGB, NB - ch * NGB)
                    n = nblk * 128
                    dv = bass.AP(dest.tensor,
                                 dest[:].offset + ch * NGB * C,
                                 [[dest[:].ap[0][0], 128], [C, nblk], [1, C]])
                    _dma_gather_raw(
                        nc.gpsimd, dv, tview,
                        W_all[:, t * 160 + ch * WCOLS: t * 160 + ch * WCOLS + (n // 16)],
                        n, nreg[n], C, queue_num=ch % 4)

            def theta_relu(t, bds, K, dest):
                """dest (128, NB*K) bf16 = relu(dnT_t.T @ block dirs)."""
                n = NB * K
                dT = dnT_all[:, t * 128:(t + 1) * 128]
                for j in range(0, n, 512):
                    w = min(512, n - j)
                    tp = ps.tile([128, 512], F32, tag="theta")
                    nc.tensor.matmul(out=tp[:, :w], lhsT=dT,
                                     rhs=bds[:, j:j + w], start=True, stop=True)
                    nc.scalar.activation(out=dest[:, j:j + w], in_=tp[:, :w], func=AF.Relu)

            def feat_matmul(t, fmap_ap, cin, lt, wbs, nout):
                """feat psum (128, nout) = [fmap | 1] @ [w; b] for tile t."""
                ftp = ps2.tile([cin, 128], F32, tag="ftp")
                nc.tensor.transpose(out=ftp[:], in_=fmap_ap, identity=ident[:])
                nc.scalar.copy(out=lt[:cin, :], in_=ftp[:])
                fp = ps2.tile([128, nout], F32, tag="feat")
                nc.tensor.matmul(out=fp[:], lhsT=lt[:], rhs=wbs[:], start=True, stop=True)
                return fp

            # ================= pass 0: dist + topk + idx + dn + conv0 ============
            for t in range(NT):
                scr = big.tile([128, V], F32, tag="scr")
                for j in range(4):
                    nd_ps = ps.tile([128, 512], F32, tag="nd")
                    nc.tensor.matmul(out=nd_ps[:],
                                     lhsT=lhsT13[:, bass.ts(t, 128)],
                                     rhs=rhs13[:, bass.ts(j, 512)], start=True, stop=True)
                    nc.vector.tensor_scalar(
                        out=scr[:, bass.ts(j, 512)].bitcast(I32),
                        in0=nd_ps[:].bitcast(I32), scalar1=-2048,
                        scalar2=None, op0=ALU.bitwise_and)
                    nc.vector.tensor_tensor(
                        out=scr[:, bass.ts(j, 512)].bitcast(I32),
                        in0=scr[:, bass.ts(j, 512)].bitcast(I32),
                        in1=iot[:, bass.ts(j, 512)], op=ALU.bitwise_or)
                v24 = sml.tile([128, 24], F32, tag="v24")
                nc.vector.max(out=v24[:, 0:8], in_=scr[:])
                nc.vector.match_replace(out=scr[:], in_to_replace=v24[:, 0:8],
                                        in_values=scr[:], imm_value=-3.0e38)
                nc.vector.max(out=v24[:, 8:16], in_=scr[:])
                nc.vector.match_replace(out=scr[:], in_to_replace=v24[:, 8:16],
                                        in_values=scr[:], imm_value=-3.0e38)
                nc.vector.max(out=v24[:, 16:24], in_=scr[:])
                ki = sml.tile([128, 24], I32, tag="ki")
                nc.vector.tensor_scalar(out=ki[:], in0=v24[:].bitcast(I32),
                                        scalar1=0x7FF, scalar2=None, op0=ALU.bitwise_and)
                kf = sml.tile([128, 24], F32, tag="kf")
                nc.vector.tensor_copy(out=kf[:], in_=ki[:])
                # idx bounce -> wrapped int16 W
                nc.sync.dma_start(out=d1[t * 128:(t + 1) * 128, :], in_=kf[:, 1:21])
                ib = sml.tile([16, 160], F32, tag="ib")
                nc.sync.dma_start(
                    out=bass.AP(ib.tensor, ib[:].offset,
                                [[ib[:].ap[0][0], 16], [20, 8], [1, 20]]),
                    in_=bass.AP(d1[:].tensor, t * 128 * 20,
                                [[20, 16], [320, 8], [1, 20]]))
                jb = sml.tile([16, 160], F32, tag="jb")
                nc.vector.tensor_copy(
                    out=bass.AP(jb.tensor, jb[:].offset,
                                [[jb[:].ap[0][0], 16], [8, 20], [1, 8]]),
                    in_=bass.AP(ib.tensor, ib[:].offset,
                                [[ib[:].ap[0][0], 16], [1, 20], [20, 8]]))
                wp = ps2.tile([128, 160], F32, tag="wp")
                nc.tensor.matmul(out=wp[:], lhsT=rsel[:], rhs=jb[:], start=True, stop=True)
                nc.scalar.copy(out=W_all[:, t * 160:(t + 1) * 160], in_=wp[:])

                # verts of this tile + bulk-gathered neighbor verts
                vt = vts[:, t * 3:(t + 1) * 3]
                nc.sync.dma_start(out=vt, in_=src[t * 128:(t + 1) * 128, :])
                vg = mid.tile([128, NB * 16], F32, tag="vg")
                gather(t, tv16, 16, vg)
                vgv = bass.AP(vg.tensor, vg[:].offset,
                              [[vg[:].ap[0][0], 128], [16, NB], [1, 3]])
                dv = mid.tile([128, NB * 3], F32, tag="dv")
                vt_b = bass.AP(vts[:].tensor, vts[:].offset + t * 3,
                               [[NT * 3, 128], [0, NB], [1, 3]])
                nc.vector.tensor_tensor(out=dv[:], in0=vgv, in1=vt_b, op=ALU.subtract)
                dsq = mid.tile([128, NB * 3], F32, tag="dsq")
                nc.vector.tensor_mul(out=dsq[:], in0=dv[:], in1=dv[:])
                nsq = sml.tile([128, NB], F32, tag="nsq")
                nc.vector.tensor_reduce(
                    out=nsq[:], in_=dsq[:].rearrange("p (r d) -> p r d", r=NB, d=3),
                    axis=mybir.AxisListType.X, op=ALU.add)
                rn = sml.tile([128, NB], F32, tag="rn")
                nc.scalar.activation(out=rn[:], in_=nsq[:], func=AF.Sqrt)
                nc.vector.tensor_scalar_max(rn[:], rn[:], 1e-12)
                nc.vector.reciprocal(out=rn[:], in_=rn[:])
                dn = mid.tile([128, NB * 3], F32, tag="dn")
                rn_b = bass.AP(rn.tensor, rn[:].offset,
                               [[rn[:].ap[0][0], 128], [1, NB], [0, 3]])
                nc.vector.tensor_tensor(out=dn[:], in0=dv[:], in1=rn_b, op=ALU.mult)
                dnp = ps2.tile([60, 128], F32, tag="ftp")
                nc.tensor.transpose(out=dnp[:], in_=dn[:, :60], identity=ident[:])
                nc.scalar.copy(out=dnT_all[:, t * 128:(t + 1) * 128], in_=dnp[:])

                # conv0: theta only -> f1
                th0 = mid.tile([128, NB * 64], BF16, tag="th")
                theta_relu(t, bd0s, 64, th0)
                mx = sml.tile([128, 64], F32, tag="mx64")
                nc.vector.tensor_reduce(
                    out=mx[:], in_=bass.AP(th0.tensor, th0[:].offset,
                                           [[th0[:].ap[0][0], 128], [1, 64], [64, NB]]),
                    axis=mybir.AxisListType.X, op=ALU.max)
                f1t = f1_all[:, t * 16:(t + 1) * 16]
                nc.vector.tensor_reduce(
                    out=f1t, in_=bass.AP(mx.tensor, mx[:].offset,
                                         [[mx[:].ap[0][0], 128], [1, 16], [16, 4]]),
                    axis=mybir.AxisListType.X, op=ALU.add)
                nc.vector.tensor_scalar_max(f1t, f1t, 0.0)
                # conv1 feature table + cached center
                fp = feat_matmul(t, f1t, 16, lt1, wb1s[:], 160)
                nc.scalar.copy(out=cen1[:, t * 32:(t + 1) * 32], in_=fp[:, 0:32])
                sup = sml.tile([128, 128], BF16, tag="sup1")
                nc.scalar.copy(out=sup[:], in_=fp[:, 32:160])
                nc.sync.dma_start(out=t1[t * 128:(t + 1) * 128, :], in_=sup[:])

            # ================= pass 1: conv1 -> f2, adain stats =================
            for t in range(NT):
                sg = mid.tile([128, NB * 128], BF16, tag="sg")
                gather(t, t1[:], 128, sg)
                th = mid.tile([128, NB * 128], BF16, tag="th")
                theta_relu(t, bd1s, 128, th)
                nc.vector.tensor_mul(out=th[:], in0=th[:], in1=sg[:])
                mx = sml.tile([128, 128], F32, tag="mx128")
                nc.vector.tensor_reduce(
                    out=mx[:], in_=bass.AP(th.tensor, th[:].offset,
                                           [[th[:].ap[0][0], 128], [1, 128], [128, NB]]),
                    axis=mybir.AxisListType.X, op=ALU.max)
                acc = sml.tile([128, 32], F32, tag="acc32")
                nc.vector.tensor_reduce(
                    out=acc[:], in_=bass.AP(mx.tensor, mx[:].offset,
                                            [[mx[:].ap[0][0], 128], [1, 32], [32, 4]]),
                    axis=mybir.AxisListType.X, op=ALU.add)
                f2t = f2_all[:, t * 32:(t + 1) * 32]
                nc.vector.tensor_add(out=acc[:], in0=acc[:], in1=cen1[:, t * 32:(t + 1) * 32])
                nc.scalar.activation(out=f2t, in_=acc[:], func=AF.Relu)
                # adain stats
                sp = ps2.tile([1, 64], F32, tag="sp")
                nc.tensor.matmul(out=sp[:, 0:32], lhsT=ones128[:], rhs=f2t, start=True, stop=True)
                f2sq = sml.tile([128, 32], F32, tag="f2sq")
                nc.vector.tensor_mul(out=f2sq[:], in0=f2t, in1=f2t)
                nc.tensor.matmul(out=sp[:, 32:64], lhsT=ones128[:], rhs=f2sq[:], start=True, stop=True)
                nc.vector.tensor_add(out=s1acc[:], in0=s1acc[:], in1=sp[:, 0:32])
                nc.vector.tensor_add(out=s2acc[:], in0=s2acc[:], in1=sp[:, 32:64])

            # ---- adain finalize ----
            stat = keep.tile([1, 64], F32)
            nc.vector.tensor_scalar_mul(stat[:, 0:32], s1acc[:], 1.0 / V)
            m2 = keep.tile([1, 32], F32)
            nc.vector.tensor_mul(out=m2[:], in0=stat[:, 0:32], in1=s1acc[:])
            nc.vector.tensor_sub(out=m2[:], in0=s2acc[:], in1=m2[:])
            nc.vector.tensor_scalar_mul(m2[:], m2[:], 1.0 / (V - 1))
            nc.scalar.activation(out=m2[:], in_=m2[:], func=AF.Sqrt)
            nc.vector.tensor_scalar_add(m2[:], m2[:], 1e-8)
            nc.vector.reciprocal(out=stat[:, 32:64], in_=m2[:])
            ones1 = keep.tile([1, 128], F32)
            nc.vector.memset(ones1[:], 1.0)
            bc_ps = ps2.tile([128, 64], F32, tag="feat")
            nc.tensor.matmul(out=bc_ps[:], lhsT=ones1[:], rhs=stat[:], start=True, stop=True)
            bc = keep.tile([128, 64], F32)
            nc.scalar.copy(out=bc[:], in_=bc_ps[:])

            # ---- pass 1b: t = adain(f2), dc1 table ----
            for t in range(NT):
                tft = sml.tile([128, 10], F32, tag="tft")
                nc.sync.dma_start(out=tft[:], in_=tf[t * 128:(t + 1) * 128, :])
                hp = feat_matmul(t, tft[:], 10, lta, wbas[:], 64)
                f2t = f2_all[:, t * 32:(t + 1) * 32]
                xn = sml.tile([128, 32], F32, tag="xn")
                nc.vector.tensor_sub(out=xn[:], in0=f2t, in1=bc[:, 0:32])
                nc.vector.tensor_mul(out=xn[:], in0=xn[:], in1=bc[:, 32:64])
                g1 = sml.tile([128, 32], F32, tag="g1")
                nc.scalar.add(out=g1[:], in_=hp[:, 0:32], add=1.0)
                nc.vector.tensor_mul(out=xn[:], in0=xn[:], in1=g1[:])
                tt = t_all[:, t * 32:(t + 1) * 32]
                nc.vector.tensor_add(out=tt, in0=xn[:], in1=hp[:, 32:64])
                fp = feat_matmul(t, tt, 32, ltd1, wbd1s[:], 80)
                nc.scalar.copy(out=cend1[:, t * 16:(t + 1) * 16], in_=fp[:, 0:16])
                sup = sml.tile([128, 64], BF16, tag="sup2")
                nc.scalar.copy(out=sup[:], in_=fp[:, 16:80])
                nc.sync.dma_start(
                    out=bass.AP(t2[:].tensor, t * 128 * 128, [[128, 128], [1, 64]]),
                    in_=sup[:])

            # ================= pass 2: dc1 -> c1, dc2 table =================
            for t in range(NT):
                sg = mid.tile([128, NB * 64], BF16, tag="sg")
                gather(t, bass.AP(t2[:].tensor, 0, [[128, V], [1, 64]]), 64, sg)
                th = mid.tile([128, NB * 64], BF16, tag="th")
                theta_relu(t, bd2s, 64, th)
                nc.vector.tensor_mul(out=th[:], in0=th[:], in1=sg[:])
                mx = sml.tile([128, 64], F32, tag="mx128")
                nc.vector.tensor_reduce(
                    out=mx[:], in_=bass.AP(th.tensor, th[:].offset,
                                           [[th[:].ap[0][0], 128], [1, 64], [64, NB]]),
                    axis=mybir.AxisListType.X, op=ALU.max)
                acc = sml.tile([128, 16], F32, tag="acc16")
                nc.vector.tensor_reduce(
                    out=acc[:], in_=bass.AP(mx.tensor, mx[:].offset,
                                            [[mx[:].ap[0][0], 128], [1, 16], [16, 4]]),
                    axis=mybir.AxisListType.X, op=ALU.add)
                c1t = c1_all[:, t * 16:(t + 1) * 16]
                nc.vector.tensor_add(out=acc[:], in0=acc[:], in1=cend1[:, t * 16:(t + 1) * 16])
                nc.scalar.activation(out=c1t, in_=acc[:], func=AF.Relu)
                fp2 = feat_matmul(t, c1t, 16, ltd2, wbd2s[:], 15)
                nc.scalar.copy(out=cend2[:, t * 3:(t + 1) * 3], in_=fp2[:, 0:3])
                sup = sml.tile([128, 12], BF16, tag="sup3")
                nc.scalar.copy(out=sup[:], in_=fp2[:, 3:15])
                nc.sync.dma_start(
                    out=bass.AP(t3[:].tensor, t * 128 * 128, [[128, 128], [1, 12]]),
                    in_=sup[:])

            # ================= pass 3: dc2 -> sigmoid -> out =================
            for t in range(NT):
                sg = mid.tile([128, NB * 32], BF16, tag="sg")
                gather(t, bass.AP(t3[:].tensor, 0, [[128, V], [1, 32]]), 32, sg)
                th = mid.tile([128, NB * 12], BF16, tag="th")
                theta_relu(t, bd3s, 12, th)
                sgv = bass.AP(sg.tensor, sg[:].offset,
                              [[sg[:].ap[0][0], 128], [32, NB], [1, 12]])
                nc.vector.tensor_tensor(out=th[:], in0=th[:], in1=sgv, op=ALU.mult)
                mx = sml.tile([128, 12], F32, tag="mx12")
                nc.vector.tensor_reduce(
                    out=mx[:], in_=bass.AP(th.tensor, th[:].offset,
                                           [[th[:].ap[0][0], 128], [1, 12], [12, NB]]),
                    axis=mybir.AxisListType.X, op=ALU.max)
                acc = sml.tile([128, 3], F32, tag="acc3")
                nc.vector.tensor_reduce(
                    out=acc[:], in_=bass.AP(mx.tensor, mx[:].offset,
                                            [[mx[:].ap[0][0], 128], [1, 3], [3, 4]]),
                    axis=mybir.AxisListType.X, op=ALU.add)
                nc.vector.tensor_add(out=acc[:], in0=acc[:], in1=cend2[:, t * 3:(t + 1) * 3])
                sig = sml.tile([128, 3], F32, tag="sig")
                nc.scalar.activation(out=sig[:], in_=acc[:], func=AF.Sigmoid)
                nc.sync.dma_start(out=out[t * 128:(t + 1) * 128, :], in_=sig[:])

    _split_excess_waits(nc)
    _encode_reloads(nc)
    return nc


_NC_CACHE = None


def _host_consts(inputs):
    bf = ml_dtypes.bfloat16
    repsel = np.zeros((16, 128), np.float32)
    for p in range(128):
        repsel[p % 16, p] = 1.0
    return {
        'bd0': _block_dirs(_normalize_cols(np.asarray(inputs['conv0_dirs'])), 64).astype(bf),
        'bd1': _block_dirs(_normalize_cols(np.asarray(inputs['conv1_dirs'])), 128).astype(bf),
        'bd2': _block_dirs(_normalize_cols(np.asarray(inputs['dc1_dirs'])), 64).astype(bf),
        'bd3': _block_dirs(_normalize_cols(np.asarray(inputs['dc2_dirs'])), 12).astype(bf),
        'wb1': np.vstack([np.asarray(inputs['conv1_w']), np.asarray(inputs['conv1_b'])[None]]).astype(bf),
        'wba': np.vstack([np.asarray(inputs['adain_w']), np.asarray(inputs['adain_b'])[None]]).astype(bf),
        'wbd1': np.vstack([np.asarray(inputs['dc1_w']), np.asarray(inputs['dc1_b'])[None]]).astype(bf),
        'wbd2': np.vstack([np.asarray(inputs['dc2_w']), np.asarray(inputs['dc2_b'])[None]]).astype(bf),
        'repsel': repsel,
        'identin': np.eye(128, dtype=np.float32),
        'iota32': np.tile(np.arange(V, dtype=np.int32)[None, :], (128, 1)),
    }


def _dist_operands(x):
    bf = ml_dtypes.bfloat16
    f = np.float32
    sq = (x * x).sum(1)
    xh = x.astype(bf); xl = (x - xh.astype(f)).astype(bf)
    sqh = sq.astype(bf); sql = (sq - sqh.astype(f)).astype(bf)
    c = -sq; ch = c.astype(bf); cl = (c - ch.astype(f)).astype(bf)
    ones = np.ones(V, bf)
    x2h = (2.0 * xh.astype(f)).astype(bf)
    x2l = (2.0 * xl.astype(f)).astype(bf)
    lhsT = np.stack([*xh.T, *xl.T, *xh.T, sqh, sql, ones, ones])
    rhs = np.stack([*x2h.T, *x2h.T, *x2l.T, -ones, -ones, ch, cl])
    return lhsT.astype(bf), rhs.astype(bf)


def kernel(**inputs):
    global _NC_CACHE
    from concourse.bass_utils import run_bass_kernel_spmd

    src = np.ascontiguousarray(np.asarray(inputs['source'], dtype=np.float32))
    tf = np.ascontiguousarray(np.asarray(inputs['target_feature'], dtype=np.float32))
    consts = {k: np.ascontiguousarray(v) for k, v in _host_consts(inputs).items()}
    if _NC_CACHE is None:
        _NC_CACHE = build_kernel()
    nc = _NC_CACHE
    in_maps = []
    for b in range(B):
        l13, r13 = _dist_operands(src[b])
        in_maps.append(dict(consts, source=src[b], target_feature=tf[b],
                            lhs13i=np.ascontiguousarray(l13),
                            rhs13i=np.ascontiguousarray(r13)))
    res = run_bass_kernel_spmd(nc, in_maps, list(range(B)))
    return np.stack([res.results[b]['out'] for b in range(B)]).astype(np.float32)


if __name__ == '__main__':
    inp = dict(np.load('/root/problem/dev/inputs.npz'))
    o = kernel(**inp)
    print(o.shape, o.dtype)


# revision 20
# speedup vs baseline: 1.1694x; 1.0002x over previous
"""Trainium2 Bass kernel for nn_Autoencoder (point-cloud GNN autoencoder).

Data-parallel over batch: 8 point clouds -> 8 NeuronCores. Per core: kNN via
bf16 hi/lo-split distance matmul + OR-index-packed top-k scan on DVE, then 4
graph-conv layers with AdaIN. Neighbor features fetched with bulk dma_gather
(mlp Q7 library, wrapped-int16 indices built via a DRAM bounce + replication
matmul); all dense matmuls in bf16.
"""
import sys, types, ctypes, contextlib
sys.path.insert(0, '/opt/trn_rl_repo')

import numpy as np
import ml_dtypes
import bass_rust
from concourse import bass, mybir, bass_isa
from concourse import library_config
from concourse.tile import TileContext

B, V, NB, SUP = 8, 2048, 20, 4
NT = V // 128          # 16 point tiles per core
GC = 640              # idxs per gather chunk: 4 equal chunks on 4 queues
F32 = mybir.dt.float32
BF16 = mybir.dt.bfloat16
I32 = mybir.dt.int32
I16 = mybir.dt.int16
AF = mybir.ActivationFunctionType
ALU = mybir.AluOpType


def _split_excess_waits(nc, max_waits=1):
    """Walrus here rejects >1 sync waits per instruction; move extras onto
    NOPs on the same engine right before it."""
    for f in nc.m.functions:
        for bb in f.blocks:
            insts = list(bb.instructions)
            out = []
            for inst in insts:
                si = getattr(inst, 'sync_info', None)
                if si is not None and si.on_wait and len(si.on_wait) > max_waits:
                    waits = list(si.on_wait)
                    move, keep = waits[:-max_waits], waits[-max_waits:]
                    for w in move:
                        eng = nc.engines[inst.engine]
                        nop = eng.nop(nofuse=True)
                        ni = nop.ins
                        for f2 in nc.m.functions:
                            for bb2 in f2.blocks:
                                if ni in bb2.instructions:
                                    bb2.instructions.remove(ni)
                        ni.sync_info = bass_rust.SyncInfo(on_wait=[w], on_update=[])
                        out.append(ni)
                    si.on_wait = keep
                out.append(inst)
            bb.instructions[:] = out


def _encode_reloads(nc):
    """codegen InstPseudoReloadLibraryIndex into raw ISA bytes (walrus can't)."""
    for f in nc.m.functions:
        for bb in f.blocks:
            for pos, inst in enumerate(list(bb.instructions)):
                if isinstance(inst, bass_isa.InstPseudoReloadLibraryIndex):
                    lowered = mybir.codegen_inst_isa_one(inst, nc._state, nc.isa)
                    if not isinstance(lowered, list):
                        lowered = [lowered]
                    bb.instructions[pos:pos + 1] = list(lowered)


def _dma_gather_raw(g, out_ap, in_ap, idxs_ap, num_idxs, num_idxs_reg, elem_size,
                    queue_num=0):
    """InstDMAGatherAnt without the 256B-elem restriction (HW-validated to 64B)."""
    from concourse._compat import exact_div
    stride_bytes = in_ap.ap[0][0] * mybir.dt.size(in_ap.dtype)
    return g.add_instruction(
        mybir.InstDMAGatherAnt(
            name=g.bass.get_next_instruction_name(),
            ins=[*g.lower_ap_dma(in_ap, for_custom_bir_dma=True),
                 g.lower_ap(idxs_ap), g.lower_val_access(num_idxs_reg)],
            outs=[g.lower_ap(out_ap)],
            transpose=False, num_idxs=num_idxs, elem_size=elem_size,
            stride_bytes_256=exact_div(stride_bytes, 256), gen_mode=0,
            single_packet=True, queue_num=queue_num,
            sbuf_tokens_per_rank=0, sbuf_free_dim_per_rank=0,
            sbuf_free_dim_pad_per_rank=0, sbuf_byte_offset=0,
        ))


def _normalize_cols(d):
    n = np.sqrt((d.astype(np.float32) ** 2).sum(0))
    return (d / np.maximum(n, 1e-12)).astype(np.float32)


def _block_dirs(dirsn, K):
    """(3, K) normalized dirs -> block-diagonal (60, NB*K): row (r,d), col (r,k)."""
    bd = np.zeros((3 * NB, NB * K), np.float32)
    for r in range(NB):
        bd[3 * r:3 * r + 3, K * r:K * (r + 1)] = dirsn
    return bd


def build_kernel():
    nc = bass.Bass(num_swdge_queues=4)
    src = nc.dram_tensor("source", [V, 3], F32, kind="ExternalInput")
    tf = nc.dram_tensor("target_feature", [V, 10], F32, kind="ExternalInput")
    # host-packed constants (bf16 for matmul operands)
    bd0 = nc.dram_tensor("bd0", [60, NB * 64], BF16, kind="ExternalInput")
    bd1 = nc.dram_tensor("bd1", [60, NB * 128], BF16, kind="ExternalInput")
    bd2 = nc.dram_tensor("bd2", [60, NB * 64], BF16, kind="ExternalInput")
    bd3 = nc.dram_tensor("bd3", [60, NB * 12], BF16, kind="ExternalInput")
    wb1 = nc.dram_tensor("wb1", [17, 160], BF16, kind="ExternalInput")
    wba = nc.dram_tensor("wba", [11, 64], BF16, kind="ExternalInput")
    wbd1 = nc.dram_tensor("wbd1", [33, 80], BF16, kind="ExternalInput")
    wbd2 = nc.dram_tensor("wbd2", [17, 15], BF16, kind="ExternalInput")
    repsel = nc.dram_tensor("repsel", [16, 128], F32, kind="ExternalInput")
    lhs13i = nc.dram_tensor("lhs13i", [13, V], BF16, kind="ExternalInput")
    rhs13i = nc.dram_tensor("rhs13i", [13, V], BF16, kind="ExternalInput")
    identin = nc.dram_tensor("identin", [128, 128], F32, kind="ExternalInput")
    iota32 = nc.dram_tensor("iota32", [128, V], I32, kind="ExternalInput")
    out = nc.dram_tensor("out", [V, 3], F32, kind="ExternalOutput")
    # DRAM scratch: idx bounce + feature tables (rows = points)
    d1 = nc.dram_tensor("d1", [V, 20], F32)
    tv = nc.dram_tensor("tv", [V, 64], F32)      # padded verts (only cols 0:3)
    t1 = nc.dram_tensor("t1", [V, 128], BF16)    # conv1 supp (256B rows)
    t2 = nc.dram_tensor("t2", [V, 128], BF16)   # dc1 supp bf16 (256B pitch, 64 used)
    t3 = nc.dram_tensor("t3", [V, 128], BF16)    # dc2 supp bf16 (256B pitch, 12 used)

    NGB = GC // 128            # gather blocks (ranks) per dma_gather chunk
    NCH = (NB + NGB - 1) // NGB  # chunks per tile-layer gather (20/NGB)
    WCOLS = GC // 16           # W columns per chunk

    with TileContext(nc) as tc:
        with (
            tc.tile_pool(name="big", bufs=4) as big,
            tc.tile_pool(name="mid", bufs=3) as mid,
            tc.tile_pool(name="sml", bufs=6) as sml,
            tc.tile_pool(name="keep", bufs=1) as keep,
            tc.tile_pool(name="ps", bufs=2, space="PSUM") as ps,
            tc.tile_pool(name="ps2", bufs=1, space="PSUM") as ps2,
        ):
            nc.gpsimd.load_library(library_config.mlp)
            nreg = {n: nc.gpsimd.to_reg(n) for n in
                    sorted({min(NGB, NB - ch * NGB) * 128 for ch in range(NCH)})}
            tv16 = bass.AP(tv[:].tensor, 0, [[64, V], [1, 16]])
            t1v = None  # set below

            ident = keep.tile([128, 128], F32)
            nc.sync.dma_start(out=ident[:], in_=identin[:])
            iot = keep.tile([128, V], I32)
            nc.sync.dma_start(out=iot[:], in_=iota32[:])
            rsel = keep.tile([16, 128], F32)
            nc.sync.dma_start(out=rsel[:], in_=repsel[:])

            lhsT13 = keep.tile([13, V], BF16)
            nc.sync.dma_start(out=lhsT13[:], in_=lhs13i[:])
            rhs13 = keep.tile([13, V], BF16)
            nc.sync.dma_start(out=rhs13[:], in_=rhs13i[:])

            # padded verts table in DRAM
            nc.sync.dma_start(
                out=bass.AP(tv[:].tensor, 0, [[64, V], [1, 3]]),
                in_=bass.AP(src[:].tensor, 0, [[3, V], [1, 3]]))

            # ---- weight constants ----
            wb1s = keep.tile([17, 160], BF16)
            nc.sync.dma_start(out=wb1s[:], in_=wb1[:])
            wbas = keep.tile([11, 64], BF16)
            nc.sync.dma_start(out=wbas[:], in_=wba[:])
            wbd1s = keep.tile([33, 80], BF16)
            nc.sync.dma_start(out=wbd1s[:], in_=wbd1[:])
            wbd2s = keep.tile([17, 15], BF16)
            nc.sync.dma_start(out=wbd2s[:], in_=wbd2[:])
            bd0s = keep.tile([60, NB * 64], BF16)
            nc.sync.dma_start(out=bd0s[:], in_=bd0[:])
            bd1s = keep.tile([60, NB * 128], BF16)
            nc.sync.dma_start(out=bd1s[:], in_=bd1[:])
            bd2s = keep.tile([60, NB * 64], BF16)
            nc.sync.dma_start(out=bd2s[:], in_=bd2[:])
            bd3s = keep.tile([60, NB * 12], BF16)
            nc.sync.dma_start(out=bd3s[:], in_=bd3[:])

            # ---- persistent state ----
            W_all = keep.tile([128, NT * 160], I16)   # wrapped gather idxs
            dnT_all = keep.tile([60, V], BF16)
            vts = keep.tile([128, NT * 3], F32)
            f1_all = keep.tile([128, NT * 16], F32)
            f2_all = keep.tile([128, NT * 32], F32)
            t_all = keep.tile([128, NT * 32], F32)
            c1_all = keep.tile([128, NT * 16], F32)
            cen1 = keep.tile([128, NT * 32], F32)
            cend1 = keep.tile([128, NT * 16], F32)
            cend2 = keep.tile([128, NT * 3], F32)
            s1acc = keep.tile([1, 32], F32)
            s2acc = keep.tile([1, 32], F32)
            nc.vector.memset(s1acc[:], 0.0)
            nc.vector.memset(s2acc[:], 0.0)
            ones128 = keep.tile([128, 1], F32)
            nc.vector.memset(ones128[:], 1.0)

            # persistent feat lhsT tiles with ones rows preset
            onesrow = keep.tile([1, 128], BF16)
            nc.vector.memset(onesrow[:], 1.0)
            lt1 = keep.tile([17, 128], BF16)
            nc.sync.dma_start(out=lt1[16:17, :], in_=onesrow[:])
            lta = keep.tile([11, 128], BF16)
            nc.sync.dma_start(out=lta[10:11, :], in_=onesrow[:])
            ltd1 = keep.tile([33, 128], BF16)
            nc.sync.dma_start(out=ltd1[32:33, :], in_=onesrow[:])
            ltd2 = keep.tile([17, 128], BF16)
            nc.sync.dma_start(out=ltd2[16:17, :], in_=onesrow[:])

            def gather(t, tview, C, dest):
                """dest (128, NB*C) <- table-view rows per W_all chunk idxs."""
                for ch in range(NCH):
                    nblk = min(NGB, NB - ch * NGB)
                    n = nblk * 128
                    dv = bass.AP(dest.tensor,
                                 dest[:].offset + ch * NGB * C,
                                 [[dest[:].ap[0][0], 128], [C, nblk], [1, C]])
                    _dma_gather_raw(
                        nc.gpsimd, dv, tview,
                        W_all[:, t * 160 + ch * WCOLS: t * 160 + ch * WCOLS + (n // 16)],
                        n, nreg[n], C, queue_num=ch % 4)

            def theta_relu(t, bds, K, dest):
                """dest (128, NB*K) bf16 = relu(dnT_t.T @ block dirs)."""
                n = NB * K
                dT = dnT_all[:, t * 128:(t + 1) * 128]
                for j in range(0, n, 512):
                    w = min(512, n - j)
                    tp = ps.tile([128, 512], F32, tag="theta")
                    nc.tensor.matmul(out=tp[:, :w], lhsT=dT,
                                     rhs=bds[:, j:j + w], start=True, stop=True)
                    nc.scalar.activation(out=dest[:, j:j + w], in_=tp[:, :w], func=AF.Relu)

            def feat_matmul(t, fmap_ap, cin, lt, wbs, nout):
                """feat psum (128, nout) = [fmap | 1] @ [w; b] for tile t."""
                ftp = ps2.tile([cin, 128], F32, tag="ftp")
                nc.tensor.transpose(out=ftp[:], in_=fmap_ap, identity=ident[:])
                nc.scalar.copy(out=lt[:cin, :], in_=ftp[:])
                fp = ps2.tile([128, nout], F32, tag="feat")
                nc.tensor.matmul(out=fp[:], lhsT=lt[:], rhs=wbs[:], start=True, stop=True)
                return fp

            # ================= pass 0: dist + topk + idx + dn + conv0 ============
            for t in range(NT):
                scr = big.tile([128, V], F32, tag="scr")
                for j in range(4):
                    nd_ps = ps.tile([128, 512], F32, tag="nd")
                    nc.tensor.matmul(out=nd_ps[:],
                                     lhsT=lhsT13[:, bass.ts(t, 128)],
                                     rhs=rhs13[:, bass.ts(j, 512)], start=True, stop=True)
                    nc.vector.tensor_scalar(
                        out=scr[:, bass.ts(j, 512)].bitcast(I32),
                        in0=nd_ps[:].bitcast(I32), scalar1=-2048,
                        scalar2=None, op0=ALU.bitwise_and)
                    nc.vector.tensor_tensor(
                        out=scr[:, bass.ts(j, 512)].bitcast(I32),
                        in0=scr[:, bass.ts(j, 512)].bitcast(I32),
                        in1=iot[:, bass.ts(j, 512)], op=ALU.bitwise_or)
                v24 = sml.tile([128, 24], F32, tag="v24")
                nc.vector.max(out=v24[:, 0:8], in_=scr[:])
                nc.vector.match_replace(out=scr[:], in_to_replace=v24[:, 0:8],
                                        in_values=scr[:], imm_value=-3.0e38)
                nc.vector.max(out=v24[:, 8:16], in_=scr[:])
                nc.vector.match_replace(out=scr[:], in_to_replace=v24[:, 8:16],
                                        in_values=scr[:], imm_value=-3.0e38)
                nc.vector.max(out=v24[:, 16:24], in_=scr[:])
                ki = sml.tile([128, 24], I32, tag="ki")
                nc.vector.tensor_scalar(out=ki[:], in0=v24[:].bitcast(I32),
                                        scalar1=0x7FF, scalar2=None, op0=ALU.bitwise_and)
                kf = sml.tile([128, 24], F32, tag="kf")
                nc.vector.tensor_copy(out=kf[:], in_=ki[:])
                # idx bounce -> wrapped int16 W
                nc.sync.dma_start(out=d1[t * 128:(t + 1) * 128, :], in_=kf[:, 1:21])
                ib = sml.tile([16, 160], F32, tag="ib")
                nc.sync.dma_start(
                    out=bass.AP(ib.tensor, ib[:].offset,
                                [[ib[:].ap[0][0], 16], [20, 8], [1, 20]]),
                    in_=bass.AP(d1[:].tensor, t * 128 * 20,
                                [[20, 16], [320, 8], [1, 20]]))
                jb = sml.tile([16, 160], F32, tag="jb")
                nc.vector.tensor_copy(
                    out=bass.AP(jb.tensor, jb[:].offset,
                                [[jb[:].ap[0][0], 16], [8, 20], [1, 8]]),
                    in_=bass.AP(ib.tensor, ib[:].offset,
                                [[ib[:].ap[0][0], 16], [1, 20], [20, 8]]))
                wp = ps2.tile([128, 160], F32, tag="wp")
                nc.tensor.matmul(out=wp[:], lhsT=rsel[:], rhs=jb[:], start=True, stop=True)
                nc.scalar.copy(out=W_all[:, t * 160:(t + 1) * 160], in_=wp[:])

                # verts of this tile + bulk-gathered neighbor verts
                vt = vts[:, t * 3:(t + 1) * 3]
                nc.sync.dma_start(out=vt, in_=src[t * 128:(t + 1) * 128, :])
                vg = mid.tile([128, NB * 16], F32, tag="vg")
                gather(t, tv16, 16, vg)
                vgv = bass.AP(vg.tensor, vg[:].offset,
                              [[vg[:].ap[0][0], 128], [16, NB], [1, 3]])
                dv = mid.tile([128, NB * 3], F32, tag="dv")
                vt_b = bass.AP(vts[:].tensor, vts[:].offset + t * 3,
                               [[NT * 3, 128], [0, NB], [1, 3]])
                nc.vector.tensor_tensor(out=dv[:], in0=vgv, in1=vt_b, op=ALU.subtract)
                dsq = mid.tile([128, NB * 3], F32, tag="dsq")
                nc.vector.tensor_mul(out=dsq[:], in0=dv[:], in1=dv[:])
                nsq = sml.tile([128, NB], F32, tag="nsq")
                nc.vector.tensor_reduce(
                    out=nsq[:], in_=dsq[:].rearrange("p (r d) -> p r d", r=NB, d=3),
                    axis=mybir.AxisListType.X, op=ALU.add)
                rn = sml.tile([128, NB], F32, tag="rn")
                nc.scalar.activation(out=rn[:], in_=nsq[:], func=AF.Sqrt)
                nc.vector.tensor_scalar_max(rn[:], rn[:], 1e-12)
                nc.vector.reciprocal(out=rn[:], in_=rn[:])
                dn = mid.tile([128, NB * 3], F32, tag="dn")
                rn_b = bass.AP(rn.tensor, rn[:].offset,
                               [[rn[:].ap[0][0], 128], [1, NB], [0, 3]])
                nc.vector.tensor_tensor(out=dn[:], in0=dv[:], in1=rn_b, op=ALU.mult)
                dnp = ps2.tile([60, 128], F32, tag="ftp")
                nc.tensor.transpose(out=dnp[:], in_=dn[:, :60], identity=ident[:])
                nc.scalar.copy(out=dnT_all[:, t * 128:(t + 1) * 128], in_=dnp[:])

                # conv0: theta only -> f1
                th0 = mid.tile([128, NB * 64], BF16, tag="th")
                theta_relu(t, bd0s, 64, th0)
                mx = sml.tile([128, 64], F32, tag="mx64")
                nc.vector.tensor_reduce(
                    out=mx[:], in_=bass.AP(th0.tensor, th0[:].offset,
                                           [[th0[:].ap[0][0], 128], [1, 64], [64, NB]]),
                    axis=mybir.AxisListType.X, op=ALU.max)
                f1t = f1_all[:, t * 16:(t + 1) * 16]
                nc.vector.tensor_reduce(
                    out=f1t, in_=bass.AP(mx.tensor, mx[:].offset,
                                         [[mx[:].ap[0][0], 128], [1, 16], [16, 4]]),
                    axis=mybir.AxisListType.X, op=ALU.add)
                nc.vector.tensor_scalar_max(f1t, f1t, 0.0)
                # conv1 feature table + cached center
                fp = feat_matmul(t, f1t, 16, lt1, wb1s[:], 160)
                nc.scalar.copy(out=cen1[:, t * 32:(t + 1) * 32], in_=fp[:, 0:32])
                sup = sml.tile([128, 128], BF16, tag="sup1")
                nc.scalar.copy(out=sup[:], in_=fp[:, 32:160])
                nc.sync.dma_start(out=t1[t * 128:(t + 1) * 128, :], in_=sup[:])

            # ================= pass 1: conv1 -> f2, adain stats =================
            for t in range(NT):
                sg = mid.tile([128, NB * 128], BF16, tag="sg")
                gather(t, t1[:], 128, sg)
                th = mid.tile([128, NB * 128], BF16, tag="th")
                theta_relu(t, bd1s, 128, th)
                nc.vector.tensor_mul(out=th[:], in0=th[:], in1=sg[:])
                mx = sml.tile([128, 128], F32, tag="mx128")
                nc.vector.tensor_reduce(
                    out=mx[:], in_=bass.AP(th.tensor, th[:].offset,
                                           [[th[:].ap[0][0], 128], [1, 128], [128, NB]]),
                    axis=mybir.AxisListType.X, op=ALU.max)
                acc = sml.tile([128, 32], F32, tag="acc32")
                nc.vector.tensor_reduce(
                    out=acc[:], in_=bass.AP(mx.tensor, mx[:].offset,
                                            [[mx[:].ap[0][0], 128], [1, 32], [32, 4]]),
                    axis=mybir.AxisListType.X, op=ALU.add)
                f2t = f2_all[:, t * 32:(t + 1) * 32]
                nc.vector.tensor_add(out=acc[:], in0=acc[:], in1=cen1[:, t * 32:(t + 1) * 32])
                nc.scalar.activation(out=f2t, in_=acc[:], func=AF.Relu)
                # adain stats
                sp = ps2.tile([1, 64], F32, tag="sp")
                nc.tensor.matmul(out=sp[:, 0:32], lhsT=ones128[:], rhs=f2t, start=True, stop=True)
                f2sq = sml.tile([128, 32], F32, tag="f2sq")
                nc.vector.tensor_mul(out=f2sq[:], in0=f2t, in1=f2t)
                nc.tensor.matmul(out=sp[:, 32:64], lhsT=ones128[:], rhs=f2sq[:], start=True, stop=True)
                nc.vector.tensor_add(out=s1acc[:], in0=s1acc[:], in1=sp[:, 0:32])
                nc.vector.tensor_add(out=s2acc[:], in0=s2acc[:], in1=sp[:, 32:64])

            # ---- adain finalize ----
            stat = keep.tile([1, 64], F32)
            nc.vector.tensor_scalar_mul(stat[:, 0:32], s1acc[:], 1.0 / V)
            m2 = keep.tile([1, 32], F32)
            nc.vector.tensor_mul(out=m2[:], in0=stat[:, 0:32], in1=s1acc[:])
            nc.vector.tensor_sub(out=m2[:], in0=s2acc[:], in1=m2[:])
            nc.vector.tensor_scalar_mul(m2[:], m2[:], 1.0 / (V - 1))
            nc.scalar.activation(out=m2[:], in_=m2[:], func=AF.Sqrt)
            nc.vector.tensor_scalar_add(m2[:], m2[:], 1e-8)
            nc.vector.reciprocal(out=stat[:, 32:64], in_=m2[:])
            ones1 = keep.tile([1, 128], F32)
            nc.vector.memset(ones1[:], 1.0)
            bc_ps = ps2.tile([128, 64], F32, tag="feat")
            nc.tensor.matmul(out=bc_ps[:], lhsT=ones1[:], rhs=stat[:], start=True, stop=True)
            bc = keep.tile([128, 64], F32)
            nc.scalar.copy(out=bc[:], in_=bc_ps[:])

            # ---- pass 1b: t = adain(f2), dc1 table ----
            for t in range(NT):
                tft = sml.tile([128, 10], F32, tag="tft")
                nc.sync.dma_start(out=tft[:], in_=tf[t * 128:(t + 1) * 128, :])
                hp = feat_matmul(t, tft[:], 10, lta, wbas[:], 64)
                f2t = f2_all[:, t * 32:(t + 1) * 32]
                xn = sml.tile([128, 32], F32, tag="xn")
                nc.vector.tensor_sub(out=xn[:], in0=f2t, in1=bc[:, 0:32])
                nc.vector.tensor_mul(out=xn[:], in0=xn[:], in1=bc[:, 32:64])
                g1 = sml.tile([128, 32], F32, tag="g1")
                nc.scalar.add(out=g1[:], in_=hp[:, 0:32], add=1.0)
                nc.vector.tensor_mul(out=xn[:], in0=xn[:], in1=g1[:])
                tt = t_all[:, t * 32:(t + 1) * 32]
                nc.vector.tensor_add(out=tt, in0=xn[:], in1=hp[:, 32:64])
                fp = feat_matmul(t, tt, 32, ltd1, wbd1s[:], 80)
                nc.scalar.copy(out=cend1[:, t * 16:(t + 1) * 16], in_=fp[:, 0:16])
                sup = sml.tile([128, 64], BF16, tag="sup2")
                nc.scalar.copy(out=sup[:], in_=fp[:, 16:80])
                nc.sync.dma_start(
                    out=bass.AP(t2[:].tensor, t * 128 * 128, [[128, 128], [1, 64]]),
                    in_=sup[:])

            # ================= pass 2: dc1 -> c1, dc2 table =================
            for t in range(NT):
                sg = mid.tile([128, NB * 64], BF16, tag="sg")
                gather(t, bass.AP(t2[:].tensor, 0, [[128, V], [1, 64]]), 64, sg)
                th = mid.tile([128, NB * 64], BF16, tag="th")
                theta_relu(t, bd2s, 64, th)
                nc.vector.tensor_mul(out=th[:], in0=th[:], in1=sg[:])
                mx = sml.tile([128, 64], F32, tag="mx128")
                nc.vector.tensor_reduce(
                    out=mx[:], in_=bass.AP(th.tensor, th[:].offset,
                                           [[th[:].ap[0][0], 128], [1, 64], [64, NB]]),
                    axis=mybir.AxisListType.X, op=ALU.max)
                acc = sml.tile([128, 16], F32, tag="acc16")
                nc.vector.tensor_reduce(
                    out=acc[:], in_=bass.AP(mx.tensor, mx[:].offset,
                                            [[mx[:].ap[0][0], 128], [1, 16], [16, 4]]),
                    axis=mybir.AxisListType.X, op=ALU.add)
                c1t = c1_all[:, t * 16:(t + 1) * 16]
                nc.vector.tensor_add(out=acc[:], in0=acc[:], in1=cend1[:, t * 16:(t + 1) * 16])
                nc.scalar.activation(out=c1t, in_=acc[:], func=AF.Relu)
                fp2 = feat_matmul(t, c1t, 16, ltd2, wbd2s[:], 15)
                nc.scalar.copy(out=cend2[:, t * 3:(t + 1) * 3], in_=fp2[:, 0:3])
                sup = sml.tile([128, 12], BF16, tag="sup3")
                nc.scalar.copy(out=sup[:], in_=fp2[:, 3:15])
                nc.sync.dma_start(
                    out=bass.AP(t3[:].tensor, t * 128 * 128, [[128, 128], [1, 12]]),
                    in_=sup[:])

            # ================= pass 3: dc2 -> sigmoid -> out =================
            for t in range(NT):
                sg = mid.tile([128, NB * 32], BF16, tag="sg")
                gather(t, bass.AP(t3[:].tensor, 0, [[128, V], [1, 32]]), 32, sg)
                th = mid.tile([128, NB * 12], BF16, tag="th")
                theta_relu(t, bd3s, 12, th)
                sgv = bass.AP(sg.tensor, sg[:].offset,
                              [[sg[:].ap[0][0], 128], [32, NB], [1, 12]])
                nc.vector.tensor_tensor(out=th[:], in0=th[:], in1=sgv, op=ALU.mult)
                mx = sml.tile([128, 12], F32, tag="mx12")
                nc.vector.tensor_reduce(
                    out=mx[:], in_=bass.AP(th.tensor, th[:].offset,
                                           [[th[:].ap[0][0], 128], [1, 12], [12, NB]]),
                    axis=mybir.AxisListType.X, op=ALU.max)
                acc = sml.tile([128, 3], F32, tag="acc3")
                nc.vector.tensor_reduce(
                    out=acc[:], in_=bass.AP(mx.tensor, mx[:].offset,
                                            [[mx[:].ap[0][0], 128], [1, 3], [3, 4]]),
                    axis=mybir.AxisListType.X, op=ALU.add)
                nc.vector.tensor_add(out=acc[:], in0=acc[:], in1=cend2[:, t * 3:(t + 1) * 3])
                sig = sml.tile([128, 3], F32, tag="sig")
                nc.scalar.activation(out=sig[:], in_=acc[:], func=AF.Sigmoid)
                nc.sync.dma_start(out=out[t * 128:(t + 1) * 128, :], in_=sig[:])

    _split_excess_waits(nc)
    _encode_reloads(nc)
    return nc


_NC_CACHE = None


def _host_consts(inputs):
    bf = ml_dtypes.bfloat16
    repsel = np.zeros((16, 128), np.float32)
    for p in range(128):
        repsel[p % 16, p] = 1.0
    return {
        'bd0': _block_dirs(_normalize_cols(np.asarray(inputs['conv0_dirs'])), 64).astype(bf),
        'bd1': _block_dirs(_normalize_cols(np.asarray(inputs['conv1_dirs'])), 128).astype(bf),
        'bd2': _block_dirs(_normalize_cols(np.asarray(inputs['dc1_dirs'])), 64).astype(bf),
        'bd3': _block_dirs(_normalize_cols(np.asarray(inputs['dc2_dirs'])), 12).astype(bf),
        'wb1': np.vstack([np.asarray(inputs['conv1_w']), np.asarray(inputs['conv1_b'])[None]]).astype(bf),
        'wba': np.vstack([np.asarray(inputs['adain_w']), np.asarray(inputs['adain_b'])[None]]).astype(bf),
        'wbd1': np.vstack([np.asarray(inputs['dc1_w']), np.asarray(inputs['dc1_b'])[None]]).astype(bf),
        'wbd2': np.vstack([np.asarray(inputs['dc2_w']), np.asarray(inputs['dc2_b'])[None]]).astype(bf),
        'repsel': repsel,
        'identin': np.eye(128, dtype=np.float32),
        'iota32': np.tile(np.arange(V, dtype=np.int32)[None, :], (128, 1)),
    }


def _dist_operands(x):
    bf = ml_dtypes.bfloat16
    f = np.float32
    sq = (x * x).sum(1)
    xh = x.astype(bf); xl = (x - xh.astype(f)).astype(bf)
    sqh = sq.astype(bf); sql = (sq - sqh.astype(f)).astype(bf)
    c = -sq; ch = c.astype(bf); cl = (c - ch.astype(f)).astype(bf)
    ones = np.ones(V, bf)
    x2h = (2.0 * xh.astype(f)).astype(bf)
    x2l = (2.0 * xl.astype(f)).astype(bf)
    lhsT = np.stack([*xh.T, *xl.T, *xh.T, sqh, sql, ones, ones])
    rhs = np.stack([*x2h.T, *x2h.T, *x2l.T, -ones, -ones, ch, cl])
    return lhsT.astype(bf), rhs.astype(bf)


def kernel(**inputs):
    global _NC_CACHE
    from concourse.bass_utils import run_bass_kernel_spmd

    src = np.ascontiguousarray(np.asarray(inputs['source'], dtype=np.float32))
    tf = np.ascontiguousarray(np.asarray(inputs['target_feature'], dtype=np.float32))
    consts = {k: np.ascontiguousarray(v) for k, v in _host_consts(inputs).items()}
    if _NC_CACHE is None:
        _NC_CACHE = build_kernel()
    nc = _NC_CACHE
    in_maps = []
    for b in range(B):
        l13, r13 = _dist_operands(src[b])
        in_maps.append(dict(consts, source=src[b], target_feature=tf[b],
                            lhs13i=np.ascontiguousarray(l13),
                            rhs13i=np.ascontiguousarray(r13)))
    res = run_bass_kernel_spmd(nc, in_maps, list(range(B)))
    return np.stack([res.results[b]['out'] for b in range(B)]).astype(np.float32)


if __name__ == '__main__':
    inp = dict(np.load('/root/problem/dev/inputs.npz'))
    o = kernel(**inp)
    print(o.shape, o.dtype)


# revision 21
# speedup vs baseline: 1.2138x; 1.0379x over previous
"""Trainium2 Bass kernel for nn_Autoencoder (point-cloud GNN autoencoder).

Data-parallel over batch: 8 point clouds -> 8 NeuronCores. Per core: kNN via
bf16 hi/lo-split distance matmul + OR-index-packed top-k scan on DVE, then 4
graph-conv layers with AdaIN. Neighbor features fetched with bulk dma_gather
(mlp Q7 library, wrapped-int16 indices built via a DRAM bounce + replication
matmul); all dense matmuls in bf16.
"""
import sys, types, ctypes, contextlib
sys.path.insert(0, '/opt/trn_rl_repo')

import numpy as np
import ml_dtypes
import bass_rust
from concourse import bass, mybir, bass_isa
from concourse import library_config
from concourse.tile import TileContext

B, V, NB, SUP = 8, 2048, 20, 4
NT = V // 128          # 16 point tiles per core
GC = 640              # idxs per gather chunk: 4 equal chunks on 4 queues
F32 = mybir.dt.float32
BF16 = mybir.dt.bfloat16
I32 = mybir.dt.int32
I16 = mybir.dt.int16
AF = mybir.ActivationFunctionType
ALU = mybir.AluOpType


def _split_excess_waits(nc, max_waits=1):
    """Walrus here rejects >1 sync waits per instruction; move extras onto
    NOPs on the same engine right before it."""
    for f in nc.m.functions:
        for bb in f.blocks:
            insts = list(bb.instructions)
            out = []
            for inst in insts:
                si = getattr(inst, 'sync_info', None)
                if si is not None and si.on_wait and len(si.on_wait) > max_waits:
                    waits = list(si.on_wait)
                    move, keep = waits[:-max_waits], waits[-max_waits:]
                    for w in move:
                        eng = nc.engines[inst.engine]
                        nop = eng.nop(nofuse=True)
                        ni = nop.ins
                        for f2 in nc.m.functions:
                            for bb2 in f2.blocks:
                                if ni in bb2.instructions:
                                    bb2.instructions.remove(ni)
                        ni.sync_info = bass_rust.SyncInfo(on_wait=[w], on_update=[])
                        out.append(ni)
                    si.on_wait = keep
                out.append(inst)
            bb.instructions[:] = out


def _encode_reloads(nc):
    """codegen InstPseudoReloadLibraryIndex into raw ISA bytes (walrus can't)."""
    for f in nc.m.functions:
        for bb in f.blocks:
            for pos, inst in enumerate(list(bb.instructions)):
                if isinstance(inst, bass_isa.InstPseudoReloadLibraryIndex):
                    lowered = mybir.codegen_inst_isa_one(inst, nc._state, nc.isa)
                    if not isinstance(lowered, list):
                        lowered = [lowered]
                    bb.instructions[pos:pos + 1] = list(lowered)


def _dma_gather_raw(g, out_ap, in_ap, idxs_ap, num_idxs, num_idxs_reg, elem_size,
                    queue_num=0):
    """InstDMAGatherAnt without the 256B-elem restriction (HW-validated to 64B)."""
    from concourse._compat import exact_div
    stride_bytes = in_ap.ap[0][0] * mybir.dt.size(in_ap.dtype)
    return g.add_instruction(
        mybir.InstDMAGatherAnt(
            name=g.bass.get_next_instruction_name(),
            ins=[*g.lower_ap_dma(in_ap, for_custom_bir_dma=True),
                 g.lower_ap(idxs_ap), g.lower_val_access(num_idxs_reg)],
            outs=[g.lower_ap(out_ap)],
            transpose=False, num_idxs=num_idxs, elem_size=elem_size,
            stride_bytes_256=exact_div(stride_bytes, 256), gen_mode=0,
            single_packet=True, queue_num=queue_num,
            sbuf_tokens_per_rank=0, sbuf_free_dim_per_rank=0,
            sbuf_free_dim_pad_per_rank=0, sbuf_byte_offset=0,
        ))


def _normalize_cols(d):
    n = np.sqrt((d.astype(np.float32) ** 2).sum(0))
    return (d / np.maximum(n, 1e-12)).astype(np.float32)


def _block_dirs(dirsn, K):
    """(3, K) normalized dirs -> block-diagonal (60, NB*K): row (r,d), col (r,k)."""
    bd = np.zeros((3 * NB, NB * K), np.float32)
    for r in range(NB):
        bd[3 * r:3 * r + 3, K * r:K * (r + 1)] = dirsn
    return bd


def build_kernel():
    nc = bass.Bass(num_swdge_queues=4)
    src = nc.dram_tensor("source", [V, 3], F32, kind="ExternalInput")
    tf = nc.dram_tensor("target_feature", [V, 10], F32, kind="ExternalInput")
    # host-packed constants (bf16 for matmul operands)
    bd0 = nc.dram_tensor("bd0", [60, NB * 64], BF16, kind="ExternalInput")
    bd1 = nc.dram_tensor("bd1", [60, NB * 128], BF16, kind="ExternalInput")
    bd2 = nc.dram_tensor("bd2", [60, NB * 64], BF16, kind="ExternalInput")
    bd3 = nc.dram_tensor("bd3", [60, NB * 12], BF16, kind="ExternalInput")
    wb1 = nc.dram_tensor("wb1", [17, 160], BF16, kind="ExternalInput")
    wba = nc.dram_tensor("wba", [11, 64], BF16, kind="ExternalInput")
    wbd1 = nc.dram_tensor("wbd1", [33, 80], BF16, kind="ExternalInput")
    wbd2 = nc.dram_tensor("wbd2", [17, 15], BF16, kind="ExternalInput")
    repsel = nc.dram_tensor("repsel", [16, 128], F32, kind="ExternalInput")
    lhs13i = nc.dram_tensor("lhs13i", [13, V], BF16, kind="ExternalInput")
    rhs13i = nc.dram_tensor("rhs13i", [13, V], BF16, kind="ExternalInput")
    identin = nc.dram_tensor("identin", [128, 128], F32, kind="ExternalInput")
    iota32 = nc.dram_tensor("iota32", [128, V], I32, kind="ExternalInput")
    out = nc.dram_tensor("out", [V, 3], F32, kind="ExternalOutput")
    # DRAM scratch: idx bounce + feature tables (rows = points)
    d1 = nc.dram_tensor("d1", [V, 20], F32)
    tv = nc.dram_tensor("tv", [V, 64], F32)      # padded verts (only cols 0:3)
    t1 = nc.dram_tensor("t1", [V, 128], BF16)    # conv1 supp (256B rows)
    t2 = nc.dram_tensor("t2", [V, 128], BF16)   # dc1 supp bf16 (256B pitch, 64 used)
    t3 = nc.dram_tensor("t3", [V, 128], BF16)    # dc2 supp bf16 (256B pitch, 12 used)

    NGB = GC // 128            # gather blocks (ranks) per dma_gather chunk
    NCH = (NB + NGB - 1) // NGB  # chunks per tile-layer gather (20/NGB)
    WCOLS = GC // 16           # W columns per chunk

    with TileContext(nc) as tc:
        with (
            tc.tile_pool(name="big", bufs=4) as big,
            tc.tile_pool(name="mid", bufs=3) as mid,
            tc.tile_pool(name="sml", bufs=6) as sml,
            tc.tile_pool(name="keep", bufs=1) as keep,
            tc.tile_pool(name="ps", bufs=2, space="PSUM") as ps,
            tc.tile_pool(name="ps2", bufs=1, space="PSUM") as ps2,
        ):
            nc.gpsimd.load_library(library_config.mlp)
            nreg = {n: nc.gpsimd.to_reg(n) for n in
                    sorted({min(NGB, NB - ch * NGB) * 128 for ch in range(NCH)})}
            tv16 = bass.AP(tv[:].tensor, 0, [[64, V], [1, 16]])
            t1v = None  # set below

            ident = keep.tile([128, 128], F32)
            nc.sync.dma_start(out=ident[:], in_=identin[:])
            iot = keep.tile([128, V], I32)
            nc.sync.dma_start(out=iot[:], in_=iota32[:])
            rsel = keep.tile([16, 128], F32)
            nc.sync.dma_start(out=rsel[:], in_=repsel[:])

            lhsT13 = keep.tile([13, V], BF16)
            nc.sync.dma_start(out=lhsT13[:], in_=lhs13i[:])
            rhs13 = keep.tile([13, V], BF16)
            nc.sync.dma_start(out=rhs13[:], in_=rhs13i[:])

            # padded verts table in DRAM
            nc.sync.dma_start(
                out=bass.AP(tv[:].tensor, 0, [[64, V], [1, 3]]),
                in_=bass.AP(src[:].tensor, 0, [[3, V], [1, 3]]))

            # ---- weight constants ----
            wb1s = keep.tile([17, 160], BF16)
            nc.sync.dma_start(out=wb1s[:], in_=wb1[:])
            wbas = keep.tile([11, 64], BF16)
            nc.sync.dma_start(out=wbas[:], in_=wba[:])
            wbd1s = keep.tile([33, 80], BF16)
            nc.sync.dma_start(out=wbd1s[:], in_=wbd1[:])
            wbd2s = keep.tile([17, 15], BF16)
            nc.sync.dma_start(out=wbd2s[:], in_=wbd2[:])
            bd0s = keep.tile([60, NB * 64], BF16)
            nc.sync.dma_start(out=bd0s[:], in_=bd0[:])
            bd1s = keep.tile([60, NB * 128], BF16)
            nc.sync.dma_start(out=bd1s[:], in_=bd1[:])
            bd2s = keep.tile([60, NB * 64], BF16)
            nc.sync.dma_start(out=bd2s[:], in_=bd2[:])
            bd3s = keep.tile([60, NB * 12], BF16)
            nc.sync.dma_start(out=bd3s[:], in_=bd3[:])

            # ---- persistent state ----
            W_all = keep.tile([128, NT * 160], I16)   # wrapped gather idxs
            dnT_all = keep.tile([60, V], BF16)
            vts = keep.tile([128, NT * 3], F32)
            f1_all = keep.tile([128, NT * 16], F32)
            f2_all = keep.tile([128, NT * 32], F32)
            t_all = keep.tile([128, NT * 32], F32)
            c1_all = keep.tile([128, NT * 16], F32)
            cen1 = keep.tile([128, NT * 32], F32)
            cend1 = keep.tile([128, NT * 16], F32)
            cend2 = keep.tile([128, NT * 3], F32)
            s1acc = keep.tile([1, 32], F32)
            s2acc = keep.tile([1, 32], F32)
            nc.vector.memset(s1acc[:], 0.0)
            nc.vector.memset(s2acc[:], 0.0)
            ones128 = keep.tile([128, 1], F32)
            nc.vector.memset(ones128[:], 1.0)

            # persistent feat lhsT tiles with ones rows preset
            onesrow = keep.tile([1, 128], BF16)
            nc.vector.memset(onesrow[:], 1.0)
            lt1 = keep.tile([17, 128], BF16)
            nc.sync.dma_start(out=lt1[16:17, :], in_=onesrow[:])
            lta = keep.tile([11, 128], BF16)
            nc.sync.dma_start(out=lta[10:11, :], in_=onesrow[:])
            ltd1 = keep.tile([33, 128], BF16)
            nc.sync.dma_start(out=ltd1[32:33, :], in_=onesrow[:])
            ltd2 = keep.tile([17, 128], BF16)
            nc.sync.dma_start(out=ltd2[16:17, :], in_=onesrow[:])

            def gather(t, tview, C, dest):
                """dest (128, NB*C) <- table-view rows per W_all chunk idxs."""
                for ch in range(NCH):
                    nblk = min(NGB, NB - ch * NGB)
                    n = nblk * 128
                    dv = bass.AP(dest.tensor,
                                 dest[:].offset + ch * NGB * C,
                                 [[dest[:].ap[0][0], 128], [C, nblk], [1, C]])
                    _dma_gather_raw(
                        nc.gpsimd, dv, tview,
                        W_all[:, t * 160 + ch * WCOLS: t * 160 + ch * WCOLS + (n // 16)],
                        n, nreg[n], C, queue_num=ch % 4)

            def theta_relu(t, bds, K, dest):
                """dest (128, NB*K) bf16 = relu(dnT_t.T @ block dirs)."""
                n = NB * K
                dT = dnT_all[:, t * 128:(t + 1) * 128]
                for j in range(0, n, 512):
                    w = min(512, n - j)
                    tp = ps.tile([128, 512], F32, tag="theta")
                    nc.tensor.matmul(out=tp[:, :w], lhsT=dT,
                                     rhs=bds[:, j:j + w], start=True, stop=True)
                    nc.scalar.activation(out=dest[:, j:j + w], in_=tp[:, :w], func=AF.Relu)

            def feat_matmul(t, fmap_ap, cin, lt, wbs, nout):
                """feat psum (128, nout) = [fmap | 1] @ [w; b] for tile t."""
                ftp = ps2.tile([cin, 128], F32, tag="ftp")
                nc.tensor.transpose(out=ftp[:], in_=fmap_ap, identity=ident[:])
                nc.scalar.copy(out=lt[:cin, :], in_=ftp[:])
                fp = ps2.tile([128, nout], F32, tag="feat")
                nc.tensor.matmul(out=fp[:], lhsT=lt[:], rhs=wbs[:], start=True, stop=True)
                return fp

            # ================= pass 0: dist + topk + idx + dn + conv0 ============
            for t in range(NT):
                scr = big.tile([128, V], F32, tag="scr")
                for j in range(4):
                    nd_ps = ps.tile([128, 512], F32, tag="nd")
                    nc.tensor.matmul(out=nd_ps[:],
                                     lhsT=lhsT13[:, bass.ts(t, 128)],
                                     rhs=rhs13[:, bass.ts(j, 512)], start=True, stop=True)
                    nc.vector.tensor_scalar(
                        out=scr[:, bass.ts(j, 512)].bitcast(I32),
                        in0=nd_ps[:].bitcast(I32), scalar1=-2048,
                        scalar2=None, op0=ALU.bitwise_and)
                    nc.vector.tensor_tensor(
                        out=scr[:, bass.ts(j, 512)].bitcast(I32),
                        in0=scr[:, bass.ts(j, 512)].bitcast(I32),
                        in1=iot[:, bass.ts(j, 512)], op=ALU.bitwise_or)
                pm = big.tile([128, 1024], F32, tag="pm")
                nc.vector.tensor_tensor(
                    out=pm[:],
                    in0=bass.AP(scr.tensor, scr[:].offset, [[scr[:].ap[0][0], 128], [2, 1024]]),
                    in1=bass.AP(scr.tensor, scr[:].offset + 1, [[scr[:].ap[0][0], 128], [2, 1024]]),
                    op=ALU.max)
                v24 = sml.tile([128, 24], F32, tag="v24")
                nc.vector.max(out=v24[:, 0:8], in_=pm[:])
                nc.vector.match_replace(out=pm[:], in_to_replace=v24[:, 0:8],
                                        in_values=pm[:], imm_value=-3.0e38)
                nc.vector.max(out=v24[:, 8:16], in_=pm[:])
                nc.vector.match_replace(out=pm[:], in_to_replace=v24[:, 8:16],
                                        in_values=pm[:], imm_value=-3.0e38)
                nc.vector.max(out=v24[:, 16:24], in_=pm[:])
                ki = sml.tile([128, 24], I32, tag="ki")
                nc.vector.tensor_scalar(out=ki[:], in0=v24[:].bitcast(I32),
                                        scalar1=0x7FF, scalar2=None, op0=ALU.bitwise_and)
                kf = sml.tile([128, 24], F32, tag="kf")
                nc.vector.tensor_copy(out=kf[:], in_=ki[:])
                # idx bounce -> wrapped int16 W
                nc.sync.dma_start(out=d1[t * 128:(t + 1) * 128, :], in_=kf[:, 1:21])
                ib = sml.tile([16, 160], F32, tag="ib")
                nc.sync.dma_start(
                    out=bass.AP(ib.tensor, ib[:].offset,
                                [[ib[:].ap[0][0], 16], [20, 8], [1, 20]]),
                    in_=bass.AP(d1[:].tensor, t * 128 * 20,
                                [[20, 16], [320, 8], [1, 20]]))
                jb = sml.tile([16, 160], F32, tag="jb")
                nc.vector.tensor_copy(
                    out=bass.AP(jb.tensor, jb[:].offset,
                                [[jb[:].ap[0][0], 16], [8, 20], [1, 8]]),
                    in_=bass.AP(ib.tensor, ib[:].offset,
                                [[ib[:].ap[0][0], 16], [1, 20], [20, 8]]))
                wp = ps2.tile([128, 160], F32, tag="wp")
                nc.tensor.matmul(out=wp[:], lhsT=rsel[:], rhs=jb[:], start=True, stop=True)
                nc.scalar.copy(out=W_all[:, t * 160:(t + 1) * 160], in_=wp[:])

                # verts of this tile + bulk-gathered neighbor verts
                vt = vts[:, t * 3:(t + 1) * 3]
                nc.sync.dma_start(out=vt, in_=src[t * 128:(t + 1) * 128, :])
                vg = mid.tile([128, NB * 16], F32, tag="vg")
                gather(t, tv16, 16, vg)
                vgv = bass.AP(vg.tensor, vg[:].offset,
                              [[vg[:].ap[0][0], 128], [16, NB], [1, 3]])
                dv = mid.tile([128, NB * 3], F32, tag="dv")
                vt_b = bass.AP(vts[:].tensor, vts[:].offset + t * 3,
                               [[NT * 3, 128], [0, NB], [1, 3]])
                nc.vector.tensor_tensor(out=dv[:], in0=vgv, in1=vt_b, op=ALU.subtract)
                dsq = mid.tile([128, NB * 3], F32, tag="dsq")
                nc.vector.tensor_mul(out=dsq[:], in0=dv[:], in1=dv[:])
                nsq = sml.tile([128, NB], F32, tag="nsq")
                nc.vector.tensor_reduce(
                    out=nsq[:], in_=dsq[:].rearrange("p (r d) -> p r d", r=NB, d=3),
                    axis=mybir.AxisListType.X, op=ALU.add)
                rn = sml.tile([128, NB], F32, tag="rn")
                nc.scalar.activation(out=rn[:], in_=nsq[:], func=AF.Sqrt)
                nc.vector.tensor_scalar_max(rn[:], rn[:], 1e-12)
                nc.vector.reciprocal(out=rn[:], in_=rn[:])
                dn = mid.tile([128, NB * 3], F32, tag="dn")
                rn_b = bass.AP(rn.tensor, rn[:].offset,
                               [[rn[:].ap[0][0], 128], [1, NB], [0, 3]])
                nc.vector.tensor_tensor(out=dn[:], in0=dv[:], in1=rn_b, op=ALU.mult)
                dnp = ps2.tile([60, 128], F32, tag="ftp")
                nc.tensor.transpose(out=dnp[:], in_=dn[:, :60], identity=ident[:])
                nc.scalar.copy(out=dnT_all[:, t * 128:(t + 1) * 128], in_=dnp[:])

                # conv0: theta only -> f1
                th0 = mid.tile([128, NB * 64], BF16, tag="th")
                theta_relu(t, bd0s, 64, th0)
                mx = sml.tile([128, 64], F32, tag="mx64")
                nc.vector.tensor_reduce(
                    out=mx[:], in_=bass.AP(th0.tensor, th0[:].offset,
                                           [[th0[:].ap[0][0], 128], [1, 64], [64, NB]]),
                    axis=mybir.AxisListType.X, op=ALU.max)
                f1t = f1_all[:, t * 16:(t + 1) * 16]
                nc.vector.tensor_reduce(
                    out=f1t, in_=bass.AP(mx.tensor, mx[:].offset,
                                         [[mx[:].ap[0][0], 128], [1, 16], [16, 4]]),
                    axis=mybir.AxisListType.X, op=ALU.add)
                nc.vector.tensor_scalar_max(f1t, f1t, 0.0)
                # conv1 feature table + cached center
                fp = feat_matmul(t, f1t, 16, lt1, wb1s[:], 160)
                nc.scalar.copy(out=cen1[:, t * 32:(t + 1) * 32], in_=fp[:, 0:32])
                sup = sml.tile([128, 128], BF16, tag="sup1")
                nc.scalar.copy(out=sup[:], in_=fp[:, 32:160])
                nc.sync.dma_start(out=t1[t * 128:(t + 1) * 128, :], in_=sup[:])

            # ================= pass 1: conv1 -> f2, adain stats =================
            for t in range(NT):
                sg = mid.tile([128, NB * 128], BF16, tag="sg")
                gather(t, t1[:], 128, sg)
                th = mid.tile([128, NB * 128], BF16, tag="th")
                theta_relu(t, bd1s, 128, th)
                nc.vector.tensor_mul(out=th[:], in0=th[:], in1=sg[:])
                mx = sml.tile([128, 128], F32, tag="mx128")
                nc.vector.tensor_reduce(
                    out=mx[:], in_=bass.AP(th.tensor, th[:].offset,
                                           [[th[:].ap[0][0], 128], [1, 128], [128, NB]]),
                    axis=mybir.AxisListType.X, op=ALU.max)
                acc = sml.tile([128, 32], F32, tag="acc32")
                nc.vector.tensor_reduce(
                    out=acc[:], in_=bass.AP(mx.tensor, mx[:].offset,
                                            [[mx[:].ap[0][0], 128], [1, 32], [32, 4]]),
                    axis=mybir.AxisListType.X, op=ALU.add)
                f2t = f2_all[:, t * 32:(t + 1) * 32]
                nc.vector.tensor_add(out=acc[:], in0=acc[:], in1=cen1[:, t * 32:(t + 1) * 32])
                nc.scalar.activation(out=f2t, in_=acc[:], func=AF.Relu)
                # adain stats
                sp = ps2.tile([1, 64], F32, tag="sp")
                nc.tensor.matmul(out=sp[:, 0:32], lhsT=ones128[:], rhs=f2t, start=True, stop=True)
                f2sq = sml.tile([128, 32], F32, tag="f2sq")
                nc.vector.tensor_mul(out=f2sq[:], in0=f2t, in1=f2t)
                nc.tensor.matmul(out=sp[:, 32:64], lhsT=ones128[:], rhs=f2sq[:], start=True, stop=True)
                nc.vector.tensor_add(out=s1acc[:], in0=s1acc[:], in1=sp[:, 0:32])
                nc.vector.tensor_add(out=s2acc[:], in0=s2acc[:], in1=sp[:, 32:64])

            # ---- adain finalize ----
            stat = keep.tile([1, 64], F32)
            nc.vector.tensor_scalar_mul(stat[:, 0:32], s1acc[:], 1.0 / V)
            m2 = keep.tile([1, 32], F32)
            nc.vector.tensor_mul(out=m2[:], in0=stat[:, 0:32], in1=s1acc[:])
            nc.vector.tensor_sub(out=m2[:], in0=s2acc[:], in1=m2[:])
            nc.vector.tensor_scalar_mul(m2[:], m2[:], 1.0 / (V - 1))
            nc.scalar.activation(out=m2[:], in_=m2[:], func=AF.Sqrt)
            nc.vector.tensor_scalar_add(m2[:], m2[:], 1e-8)
            nc.vector.reciprocal(out=stat[:, 32:64], in_=m2[:])
            ones1 = keep.tile([1, 128], F32)
            nc.vector.memset(ones1[:], 1.0)
            bc_ps = ps2.tile([128, 64], F32, tag="feat")
            nc.tensor.matmul(out=bc_ps[:], lhsT=ones1[:], rhs=stat[:], start=True, stop=True)
            bc = keep.tile([128, 64], F32)
            nc.scalar.copy(out=bc[:], in_=bc_ps[:])

            # ---- pass 1b: t = adain(f2), dc1 table ----
            for t in range(NT):
                tft = sml.tile([128, 10], F32, tag="tft")
                nc.sync.dma_start(out=tft[:], in_=tf[t * 128:(t + 1) * 128, :])
                hp = feat_matmul(t, tft[:], 10, lta, wbas[:], 64)
                f2t = f2_all[:, t * 32:(t + 1) * 32]
                xn = sml.tile([128, 32], F32, tag="xn")
                nc.vector.tensor_sub(out=xn[:], in0=f2t, in1=bc[:, 0:32])
                nc.vector.tensor_mul(out=xn[:], in0=xn[:], in1=bc[:, 32:64])
                g1 = sml.tile([128, 32], F32, tag="g1")
                nc.scalar.add(out=g1[:], in_=hp[:, 0:32], add=1.0)
                nc.vector.tensor_mul(out=xn[:], in0=xn[:], in1=g1[:])
                tt = t_all[:, t * 32:(t + 1) * 32]
                nc.vector.tensor_add(out=tt, in0=xn[:], in1=hp[:, 32:64])
                fp = feat_matmul(t, tt, 32, ltd1, wbd1s[:], 80)
                nc.scalar.copy(out=cend1[:, t * 16:(t + 1) * 16], in_=fp[:, 0:16])
                sup = sml.tile([128, 64], BF16, tag="sup2")
                nc.scalar.copy(out=sup[:], in_=fp[:, 16:80])
                nc.sync.dma_start(
                    out=bass.AP(t2[:].tensor, t * 128 * 128, [[128, 128], [1, 64]]),
                    in_=sup[:])

            # ================= pass 2: dc1 -> c1, dc2 table =================
            for t in range(NT):
                sg = mid.tile([128, NB * 64], BF16, tag="sg")
                gather(t, bass.AP(t2[:].tensor, 0, [[128, V], [1, 64]]), 64, sg)
                th = mid.tile([128, NB * 64], BF16, tag="th")
                theta_relu(t, bd2s, 64, th)
                nc.vector.tensor_mul(out=th[:], in0=th[:], in1=sg[:])
                mx = sml.tile([128, 64], F32, tag="mx128")
                nc.vector.tensor_reduce(
                    out=mx[:], in_=bass.AP(th.tensor, th[:].offset,
                                           [[th[:].ap[0][0], 128], [1, 64], [64, NB]]),
                    axis=mybir.AxisListType.X, op=ALU.max)
                acc = sml.tile([128, 16], F32, tag="acc16")
                nc.vector.tensor_reduce(
                    out=acc[:], in_=bass.AP(mx.tensor, mx[:].offset,
                                            [[mx[:].ap[0][0], 128], [1, 16], [16, 4]]),
                    axis=mybir.AxisListType.X, op=ALU.add)
                c1t = c1_all[:, t * 16:(t + 1) * 16]
                nc.vector.tensor_add(out=acc[:], in0=acc[:], in1=cend1[:, t * 16:(t + 1) * 16])
                nc.scalar.activation(out=c1t, in_=acc[:], func=AF.Relu)
                fp2 = feat_matmul(t, c1t, 16, ltd2, wbd2s[:], 15)
                nc.scalar.copy(out=cend2[:, t * 3:(t + 1) * 3], in_=fp2[:, 0:3])
                sup = sml.tile([128, 12], BF16, tag="sup3")
                nc.scalar.copy(out=sup[:], in_=fp2[:, 3:15])
                nc.sync.dma_start(
                    out=bass.AP(t3[:].tensor, t * 128 * 128, [[128, 128], [1, 12]]),
                    in_=sup[:])

            # ================= pass 3: dc2 -> sigmoid -> out =================
            for t in range(NT):
                sg = mid.tile([128, NB * 32], BF16, tag="sg")
                gather(t, bass.AP(t3[:].tensor, 0, [[128, V], [1, 32]]), 32, sg)
                th = mid.tile([128, NB * 12], BF16, tag="th")
                theta_relu(t, bd3s, 12, th)
                sgv = bass.AP(sg.tensor, sg[:].offset,
                              [[sg[:].ap[0][0], 128], [32, NB], [1, 12]])
                nc.vector.tensor_tensor(out=th[:], in0=th[:], in1=sgv, op=ALU.mult)
                mx = sml.tile([128, 12], F32, tag="mx12")
                nc.vector.tensor_reduce(
                    out=mx[:], in_=bass.AP(th.tensor, th[:].offset,
                                           [[th[:].ap[0][0], 128], [1, 12], [12, NB]]),
                    axis=mybir.AxisListType.X, op=ALU.max)
                acc = sml.tile([128, 3], F32, tag="acc3")
                nc.vector.tensor_reduce(
                    out=acc[:], in_=bass.AP(mx.tensor, mx[:].offset,
                                            [[mx[:].ap[0][0], 128], [1, 3], [3, 4]]),
                    axis=mybir.AxisListType.X, op=ALU.add)
                nc.vector.tensor_add(out=acc[:], in0=acc[:], in1=cend2[:, t * 3:(t + 1) * 3])
                sig = sml.tile([128, 3], F32, tag="sig")
                nc.scalar.activation(out=sig[:], in_=acc[:], func=AF.Sigmoid)
                nc.sync.dma_start(out=out[t * 128:(t + 1) * 128, :], in_=sig[:])

    _split_excess_waits(nc)
    _encode_reloads(nc)
    return nc


_NC_CACHE = None


def _host_consts(inputs):
    bf = ml_dtypes.bfloat16
    repsel = np.zeros((16, 128), np.float32)
    for p in range(128):
        repsel[p % 16, p] = 1.0
    return {
        'bd0': _block_dirs(_normalize_cols(np.asarray(inputs['conv0_dirs'])), 64).astype(bf),
        'bd1': _block_dirs(_normalize_cols(np.asarray(inputs['conv1_dirs'])), 128).astype(bf),
        'bd2': _block_dirs(_normalize_cols(np.asarray(inputs['dc1_dirs'])), 64).astype(bf),
        'bd3': _block_dirs(_normalize_cols(np.asarray(inputs['dc2_dirs'])), 12).astype(bf),
        'wb1': np.vstack([np.asarray(inputs['conv1_w']), np.asarray(inputs['conv1_b'])[None]]).astype(bf),
        'wba': np.vstack([np.asarray(inputs['adain_w']), np.asarray(inputs['adain_b'])[None]]).astype(bf),
        'wbd1': np.vstack([np.asarray(inputs['dc1_w']), np.asarray(inputs['dc1_b'])[None]]).astype(bf),
        'wbd2': np.vstack([np.asarray(inputs['dc2_w']), np.asarray(inputs['dc2_b'])[None]]).astype(bf),
        'repsel': repsel,
        'identin': np.eye(128, dtype=np.float32),
        'iota32': np.tile(np.arange(V, dtype=np.int32)[None, :], (128, 1)),
    }


def _dist_operands(x):
    bf = ml_dtypes.bfloat16
    f = np.float32
    sq = (x * x).sum(1)
    xh = x.astype(bf); xl = (x - xh.astype(f)).astype(bf)
    sqh = sq.astype(bf); sql = (sq - sqh.astype(f)).astype(bf)
    c = -sq; ch = c.astype(bf); cl = (c - ch.astype(f)).astype(bf)
    ones = np.ones(V, bf)
    x2h = (2.0 * xh.astype(f)).astype(bf)
    x2l = (2.0 * xl.astype(f)).astype(bf)
    lhsT = np.stack([*xh.T, *xl.T, *xh.T, sqh, sql, ones, ones])
    rhs = np.stack([*x2h.T, *x2h.T, *x2l.T, -ones, -ones, ch, cl])
    return lhsT.astype(bf), rhs.astype(bf)


def kernel(**inputs):
    global _NC_CACHE
    from concourse.bass_utils import run_bass_kernel_spmd

    src = np.ascontiguousarray(np.asarray(inputs['source'], dtype=np.float32))
    tf = np.ascontiguousarray(np.asarray(inputs['target_feature'], dtype=np.float32))
    consts = {k: np.ascontiguousarray(v) for k, v in _host_consts(inputs).items()}
    if _NC_CACHE is None:
        _NC_CACHE = build_kernel()
    nc = _NC_CACHE
    in_maps = []
    for b in range(B):
        l13, r13 = _dist_operands(src[b])
        in_maps.append(dict(consts, source=src[b], target_feature=tf[b],
                            lhs13i=np.ascontiguousarray(l13),
                            rhs13i=np.ascontiguousarray(r13)))
    res = run_bass_kernel_spmd(nc, in_maps, list(range(B)))
    return np.stack([res.results[b]['out'] for b in range(B)]).astype(np.float32)


if __name__ == '__main__':
    inp = dict(np.load('/root/problem/dev/inputs.npz'))
    o = kernel(**inp)
    print(o.shape, o.dtype)


# revision 22
# speedup vs baseline: 1.2423x; 1.0235x over previous
"""Trainium2 Bass kernel for nn_Autoencoder (point-cloud GNN autoencoder).

Data-parallel over batch: 8 point clouds -> 8 NeuronCores. Per core: kNN via
bf16 hi/lo-split distance matmul + OR-index-packed top-k scan on DVE, then 4
graph-conv layers with AdaIN. Neighbor features fetched with bulk dma_gather
(mlp Q7 library, wrapped-int16 indices built via a DRAM bounce + replication
matmul); all dense matmuls in bf16.
"""
import sys, types, ctypes, contextlib
sys.path.insert(0, '/opt/trn_rl_repo')

import numpy as np
import ml_dtypes
import bass_rust
from concourse import bass, mybir, bass_isa
from concourse import library_config
from concourse.tile import TileContext

B, V, NB, SUP = 8, 2048, 20, 4
NT = V // 128          # 16 point tiles per core
GC = 640              # idxs per gather chunk: 4 equal chunks on 4 queues
F32 = mybir.dt.float32
BF16 = mybir.dt.bfloat16
I32 = mybir.dt.int32
I16 = mybir.dt.int16
AF = mybir.ActivationFunctionType
ALU = mybir.AluOpType


def _split_excess_waits(nc, max_waits=1):
    """Walrus here rejects >1 sync waits per instruction; move extras onto
    NOPs on the same engine right before it."""
    for f in nc.m.functions:
        for bb in f.blocks:
            insts = list(bb.instructions)
            out = []
            for inst in insts:
                si = getattr(inst, 'sync_info', None)
                if si is not None and si.on_wait and len(si.on_wait) > max_waits:
                    waits = list(si.on_wait)
                    move, keep = waits[:-max_waits], waits[-max_waits:]
                    for w in move:
                        eng = nc.engines[inst.engine]
                        nop = eng.nop(nofuse=True)
                        ni = nop.ins
                        for f2 in nc.m.functions:
                            for bb2 in f2.blocks:
                                if ni in bb2.instructions:
                                    bb2.instructions.remove(ni)
                        ni.sync_info = bass_rust.SyncInfo(on_wait=[w], on_update=[])
                        out.append(ni)
                    si.on_wait = keep
                out.append(inst)
            bb.instructions[:] = out


def _encode_reloads(nc):
    """codegen InstPseudoReloadLibraryIndex into raw ISA bytes (walrus can't)."""
    for f in nc.m.functions:
        for bb in f.blocks:
            for pos, inst in enumerate(list(bb.instructions)):
                if isinstance(inst, bass_isa.InstPseudoReloadLibraryIndex):
                    lowered = mybir.codegen_inst_isa_one(inst, nc._state, nc.isa)
                    if not isinstance(lowered, list):
                        lowered = [lowered]
                    bb.instructions[pos:pos + 1] = list(lowered)


def _dma_gather_raw(g, out_ap, in_ap, idxs_ap, num_idxs, num_idxs_reg, elem_size,
                    queue_num=0):
    """InstDMAGatherAnt without the 256B-elem restriction (HW-validated to 64B)."""
    from concourse._compat import exact_div
    stride_bytes = in_ap.ap[0][0] * mybir.dt.size(in_ap.dtype)
    return g.add_instruction(
        mybir.InstDMAGatherAnt(
            name=g.bass.get_next_instruction_name(),
            ins=[*g.lower_ap_dma(in_ap, for_custom_bir_dma=True),
                 g.lower_ap(idxs_ap), g.lower_val_access(num_idxs_reg)],
            outs=[g.lower_ap(out_ap)],
            transpose=False, num_idxs=num_idxs, elem_size=elem_size,
            stride_bytes_256=exact_div(stride_bytes, 256), gen_mode=0,
            single_packet=True, queue_num=queue_num,
            sbuf_tokens_per_rank=0, sbuf_free_dim_per_rank=0,
            sbuf_free_dim_pad_per_rank=0, sbuf_byte_offset=0,
        ))


def _normalize_cols(d):
    n = np.sqrt((d.astype(np.float32) ** 2).sum(0))
    return (d / np.maximum(n, 1e-12)).astype(np.float32)


def _block_dirs(dirsn, K):
    """(3, K) normalized dirs -> block-diagonal (60, NB*K): row (r,d), col (r,k)."""
    bd = np.zeros((3 * NB, NB * K), np.float32)
    for r in range(NB):
        bd[3 * r:3 * r + 3, K * r:K * (r + 1)] = dirsn
    return bd


def build_kernel():
    nc = bass.Bass(num_swdge_queues=4)
    src = nc.dram_tensor("source", [V, 3], F32, kind="ExternalInput")
    tf = nc.dram_tensor("target_feature", [V, 10], F32, kind="ExternalInput")
    # host-packed constants (bf16 for matmul operands)
    bd0 = nc.dram_tensor("bd0", [60, NB * 64], BF16, kind="ExternalInput")
    bd1 = nc.dram_tensor("bd1", [60, NB * 128], BF16, kind="ExternalInput")
    bd2 = nc.dram_tensor("bd2", [60, NB * 64], BF16, kind="ExternalInput")
    bd3 = nc.dram_tensor("bd3", [60, NB * 12], BF16, kind="ExternalInput")
    wb1 = nc.dram_tensor("wb1", [17, 160], BF16, kind="ExternalInput")
    wba = nc.dram_tensor("wba", [11, 64], BF16, kind="ExternalInput")
    wbd1 = nc.dram_tensor("wbd1", [33, 80], BF16, kind="ExternalInput")
    wbd2 = nc.dram_tensor("wbd2", [17, 15], BF16, kind="ExternalInput")
    repsel = nc.dram_tensor("repsel", [16, 128], F32, kind="ExternalInput")
    lhs13i = nc.dram_tensor("lhs13i", [13, V], BF16, kind="ExternalInput")
    rhs13i = nc.dram_tensor("rhs13i", [13, V], BF16, kind="ExternalInput")
    identin = nc.dram_tensor("identin", [128, 128], F32, kind="ExternalInput")
    iota32 = nc.dram_tensor("iota32", [128, V], I32, kind="ExternalInput")
    out = nc.dram_tensor("out", [V, 3], F32, kind="ExternalOutput")
    # DRAM scratch: idx bounce + feature tables (rows = points)
    d1 = nc.dram_tensor("d1", [V, 20], F32)
    tv = nc.dram_tensor("tv", [V, 64], F32)      # padded verts (only cols 0:3)
    t1 = nc.dram_tensor("t1", [V, 128], BF16)    # conv1 supp (256B rows)
    t2 = nc.dram_tensor("t2", [V, 128], BF16)   # dc1 supp bf16 (256B pitch, 64 used)
    t3 = nc.dram_tensor("t3", [V, 128], BF16)    # dc2 supp bf16 (256B pitch, 12 used)

    NGB = GC // 128            # gather blocks (ranks) per dma_gather chunk
    NCH = (NB + NGB - 1) // NGB  # chunks per tile-layer gather (20/NGB)
    WCOLS = GC // 16           # W columns per chunk

    with TileContext(nc) as tc:
        with (
            tc.tile_pool(name="big", bufs=4) as big,
            tc.tile_pool(name="mid", bufs=4) as mid,
            tc.tile_pool(name="sml", bufs=6) as sml,
            tc.tile_pool(name="keep", bufs=1) as keep,
            tc.tile_pool(name="ps", bufs=2, space="PSUM") as ps,
            tc.tile_pool(name="ps2", bufs=1, space="PSUM") as ps2,
        ):
            nc.gpsimd.load_library(library_config.mlp)
            nreg = {n: nc.gpsimd.to_reg(n) for n in
                    sorted({min(NGB, NB - ch * NGB) * 128 for ch in range(NCH)})}
            tv16 = bass.AP(tv[:].tensor, 0, [[64, V], [1, 16]])
            t1v = None  # set below

            ident = keep.tile([128, 128], F32)
            nc.sync.dma_start(out=ident[:], in_=identin[:])
            iot = keep.tile([128, V], I32)
            nc.sync.dma_start(out=iot[:], in_=iota32[:])
            rsel = keep.tile([16, 128], F32)
            nc.sync.dma_start(out=rsel[:], in_=repsel[:])

            lhsT13 = keep.tile([13, V], BF16)
            nc.sync.dma_start(out=lhsT13[:], in_=lhs13i[:])
            rhs13 = keep.tile([13, V], BF16)
            nc.sync.dma_start(out=rhs13[:], in_=rhs13i[:])

            # padded verts table in DRAM
            nc.sync.dma_start(
                out=bass.AP(tv[:].tensor, 0, [[64, V], [1, 3]]),
                in_=bass.AP(src[:].tensor, 0, [[3, V], [1, 3]]))

            # ---- weight constants ----
            wb1s = keep.tile([17, 160], BF16)
            nc.sync.dma_start(out=wb1s[:], in_=wb1[:])
            wbas = keep.tile([11, 64], BF16)
            nc.sync.dma_start(out=wbas[:], in_=wba[:])
            wbd1s = keep.tile([33, 80], BF16)
            nc.sync.dma_start(out=wbd1s[:], in_=wbd1[:])
            wbd2s = keep.tile([17, 15], BF16)
            nc.sync.dma_start(out=wbd2s[:], in_=wbd2[:])
            bd0s = keep.tile([60, NB * 64], BF16)
            nc.sync.dma_start(out=bd0s[:], in_=bd0[:])
            bd1s = keep.tile([60, NB * 128], BF16)
            nc.sync.dma_start(out=bd1s[:], in_=bd1[:])
            bd2s = keep.tile([60, NB * 64], BF16)
            nc.sync.dma_start(out=bd2s[:], in_=bd2[:])
            bd3s = keep.tile([60, NB * 12], BF16)
            nc.sync.dma_start(out=bd3s[:], in_=bd3[:])

            # ---- persistent state ----
            W_all = keep.tile([128, NT * 160], I16)   # wrapped gather idxs
            dnT_all = keep.tile([60, V], BF16)
            vts = keep.tile([128, NT * 3], F32)
            f1_all = keep.tile([128, NT * 16], F32)
            f2_all = keep.tile([128, NT * 32], F32)
            t_all = keep.tile([128, NT * 32], F32)
            c1_all = keep.tile([128, NT * 16], F32)
            cen1 = keep.tile([128, NT * 32], F32)
            cend1 = keep.tile([128, NT * 16], F32)
            cend2 = keep.tile([128, NT * 3], F32)
            s1acc = keep.tile([1, 32], F32)
            s2acc = keep.tile([1, 32], F32)
            nc.vector.memset(s1acc[:], 0.0)
            nc.vector.memset(s2acc[:], 0.0)
            ones128 = keep.tile([128, 1], F32)
            nc.vector.memset(ones128[:], 1.0)

            # persistent feat lhsT tiles with ones rows preset
            onesrow = keep.tile([1, 128], BF16)
            nc.vector.memset(onesrow[:], 1.0)
            lt1 = keep.tile([17, 128], BF16)
            nc.sync.dma_start(out=lt1[16:17, :], in_=onesrow[:])
            lta = keep.tile([11, 128], BF16)
            nc.sync.dma_start(out=lta[10:11, :], in_=onesrow[:])
            ltd1 = keep.tile([33, 128], BF16)
            nc.sync.dma_start(out=ltd1[32:33, :], in_=onesrow[:])
            ltd2 = keep.tile([17, 128], BF16)
            nc.sync.dma_start(out=ltd2[16:17, :], in_=onesrow[:])

            def gather(t, tview, C, dest):
                """dest (128, NB*C) <- table-view rows per W_all chunk idxs."""
                for ch in range(NCH):
                    nblk = min(NGB, NB - ch * NGB)
                    n = nblk * 128
                    dv = bass.AP(dest.tensor,
                                 dest[:].offset + ch * NGB * C,
                                 [[dest[:].ap[0][0], 128], [C, nblk], [1, C]])
                    _dma_gather_raw(
                        nc.gpsimd, dv, tview,
                        W_all[:, t * 160 + ch * WCOLS: t * 160 + ch * WCOLS + (n // 16)],
                        n, nreg[n], C, queue_num=ch % 4)

            def theta_relu(t, bds, K, dest):
                """dest (128, NB*K) bf16 = relu(dnT_t.T @ block dirs)."""
                n = NB * K
                dT = dnT_all[:, t * 128:(t + 1) * 128]
                for j in range(0, n, 512):
                    w = min(512, n - j)
                    tp = ps.tile([128, 512], F32, tag="theta")
                    nc.tensor.matmul(out=tp[:, :w], lhsT=dT,
                                     rhs=bds[:, j:j + w], start=True, stop=True)
                    nc.scalar.activation(out=dest[:, j:j + w], in_=tp[:, :w], func=AF.Relu)

            def feat_matmul(t, fmap_ap, cin, lt, wbs, nout):
                """feat psum (128, nout) = [fmap | 1] @ [w; b] for tile t."""
                ftp = ps2.tile([cin, 128], F32, tag="ftp")
                nc.tensor.transpose(out=ftp[:], in_=fmap_ap, identity=ident[:])
                nc.scalar.copy(out=lt[:cin, :], in_=ftp[:])
                fp = ps2.tile([128, nout], F32, tag="feat")
                nc.tensor.matmul(out=fp[:], lhsT=lt[:], rhs=wbs[:], start=True, stop=True)
                return fp

            # ================= pass 0: dist + topk + idx + dn + conv0 ============
            for t in range(NT):
                scr = big.tile([128, V], F32, tag="scr")
                for j in range(4):
                    nd_ps = ps.tile([128, 512], F32, tag="nd")
                    nc.tensor.matmul(out=nd_ps[:],
                                     lhsT=lhsT13[:, bass.ts(t, 128)],
                                     rhs=rhs13[:, bass.ts(j, 512)], start=True, stop=True)
                    nc.vector.tensor_scalar(
                        out=scr[:, bass.ts(j, 512)].bitcast(I32),
                        in0=nd_ps[:].bitcast(I32), scalar1=-2048,
                        scalar2=None, op0=ALU.bitwise_and)
                    nc.vector.tensor_tensor(
                        out=scr[:, bass.ts(j, 512)].bitcast(I32),
                        in0=scr[:, bass.ts(j, 512)].bitcast(I32),
                        in1=iot[:, bass.ts(j, 512)], op=ALU.bitwise_or)
                pm = big.tile([128, 1024], F32, tag="pm")
                nc.vector.tensor_tensor(
                    out=pm[:],
                    in0=bass.AP(scr.tensor, scr[:].offset, [[scr[:].ap[0][0], 128], [2, 1024]]),
                    in1=bass.AP(scr.tensor, scr[:].offset + 1, [[scr[:].ap[0][0], 128], [2, 1024]]),
                    op=ALU.max)
                v24 = sml.tile([128, 24], F32, tag="v24")
                nc.vector.max(out=v24[:, 0:8], in_=pm[:])
                nc.vector.match_replace(out=pm[:], in_to_replace=v24[:, 0:8],
                                        in_values=pm[:], imm_value=-3.0e38)
                nc.vector.max(out=v24[:, 8:16], in_=pm[:])
                nc.vector.match_replace(out=pm[:], in_to_replace=v24[:, 8:16],
                                        in_values=pm[:], imm_value=-3.0e38)
                nc.vector.max(out=v24[:, 16:24], in_=pm[:])
                ki = sml.tile([128, 24], I32, tag="ki")
                nc.vector.tensor_scalar(out=ki[:], in0=v24[:].bitcast(I32),
                                        scalar1=0x7FF, scalar2=None, op0=ALU.bitwise_and)
                kf = sml.tile([128, 24], F32, tag="kf")
                nc.vector.tensor_copy(out=kf[:], in_=ki[:])
                # idx bounce -> wrapped int16 W
                nc.sync.dma_start(out=d1[t * 128:(t + 1) * 128, :], in_=kf[:, 1:21])
                ib = sml.tile([16, 160], F32, tag="ib")
                nc.sync.dma_start(
                    out=bass.AP(ib.tensor, ib[:].offset,
                                [[ib[:].ap[0][0], 16], [20, 8], [1, 20]]),
                    in_=bass.AP(d1[:].tensor, t * 128 * 20,
                                [[20, 16], [320, 8], [1, 20]]))
                jb = sml.tile([16, 160], F32, tag="jb")
                nc.vector.tensor_copy(
                    out=bass.AP(jb.tensor, jb[:].offset,
                                [[jb[:].ap[0][0], 16], [8, 20], [1, 8]]),
                    in_=bass.AP(ib.tensor, ib[:].offset,
                                [[ib[:].ap[0][0], 16], [1, 20], [20, 8]]))
                wp = ps2.tile([128, 160], F32, tag="wp")
                nc.tensor.matmul(out=wp[:], lhsT=rsel[:], rhs=jb[:], start=True, stop=True)
                nc.scalar.copy(out=W_all[:, t * 160:(t + 1) * 160], in_=wp[:])

                # verts of this tile + bulk-gathered neighbor verts
                vt = vts[:, t * 3:(t + 1) * 3]
                nc.sync.dma_start(out=vt, in_=src[t * 128:(t + 1) * 128, :])
                vg = mid.tile([128, NB * 16], F32, tag="vg")
                gather(t, tv16, 16, vg)
                vgv = bass.AP(vg.tensor, vg[:].offset,
                              [[vg[:].ap[0][0], 128], [16, NB], [1, 3]])
                dv = mid.tile([128, NB * 3], F32, tag="dv")
                vt_b = bass.AP(vts[:].tensor, vts[:].offset + t * 3,
                               [[NT * 3, 128], [0, NB], [1, 3]])
                nc.vector.tensor_tensor(out=dv[:], in0=vgv, in1=vt_b, op=ALU.subtract)
                dsq = mid.tile([128, NB * 3], F32, tag="dsq")
                nc.vector.tensor_mul(out=dsq[:], in0=dv[:], in1=dv[:])
                nsq = sml.tile([128, NB], F32, tag="nsq")
                nc.vector.tensor_reduce(
                    out=nsq[:], in_=dsq[:].rearrange("p (r d) -> p r d", r=NB, d=3),
                    axis=mybir.AxisListType.X, op=ALU.add)
                rn = sml.tile([128, NB], F32, tag="rn")
                nc.scalar.activation(out=rn[:], in_=nsq[:], func=AF.Sqrt)
                nc.vector.tensor_scalar_max(rn[:], rn[:], 1e-12)
                nc.vector.reciprocal(out=rn[:], in_=rn[:])
                dn = mid.tile([128, NB * 3], F32, tag="dn")
                rn_b = bass.AP(rn.tensor, rn[:].offset,
                               [[rn[:].ap[0][0], 128], [1, NB], [0, 3]])
                nc.vector.tensor_tensor(out=dn[:], in0=dv[:], in1=rn_b, op=ALU.mult)
                dnp = ps2.tile([60, 128], F32, tag="ftp")
                nc.tensor.transpose(out=dnp[:], in_=dn[:, :60], identity=ident[:])
                nc.scalar.copy(out=dnT_all[:, t * 128:(t + 1) * 128], in_=dnp[:])

                # conv0: theta only -> f1
                th0 = mid.tile([128, NB * 64], BF16, tag="th")
                theta_relu(t, bd0s, 64, th0)
                mx = sml.tile([128, 64], F32, tag="mx64")
                nc.vector.tensor_reduce(
                    out=mx[:], in_=bass.AP(th0.tensor, th0[:].offset,
                                           [[th0[:].ap[0][0], 128], [1, 64], [64, NB]]),
                    axis=mybir.AxisListType.X, op=ALU.max)
                f1t = f1_all[:, t * 16:(t + 1) * 16]
                nc.vector.tensor_reduce(
                    out=f1t, in_=bass.AP(mx.tensor, mx[:].offset,
                                         [[mx[:].ap[0][0], 128], [1, 16], [16, 4]]),
                    axis=mybir.AxisListType.X, op=ALU.add)
                nc.vector.tensor_scalar_max(f1t, f1t, 0.0)
                # conv1 feature table + cached center
                fp = feat_matmul(t, f1t, 16, lt1, wb1s[:], 160)
                nc.scalar.copy(out=cen1[:, t * 32:(t + 1) * 32], in_=fp[:, 0:32])
                sup = sml.tile([128, 128], BF16, tag="sup1")
                nc.scalar.copy(out=sup[:], in_=fp[:, 32:160])
                nc.sync.dma_start(out=t1[t * 128:(t + 1) * 128, :], in_=sup[:])

            # ================= pass 1: conv1 -> f2, adain stats =================
            for t in range(NT):
                sg = mid.tile([128, NB * 128], BF16, tag="sg")
                gather(t, t1[:], 128, sg)
                th = mid.tile([128, NB * 128], BF16, tag="th")
                theta_relu(t, bd1s, 128, th)
                nc.vector.tensor_mul(out=th[:], in0=th[:], in1=sg[:])
                mx = sml.tile([128, 128], F32, tag="mx128")
                nc.vector.tensor_reduce(
                    out=mx[:], in_=bass.AP(th.tensor, th[:].offset,
                                           [[th[:].ap[0][0], 128], [1, 128], [128, NB]]),
                    axis=mybir.AxisListType.X, op=ALU.max)
                acc = sml.tile([128, 32], F32, tag="acc32")
                nc.vector.tensor_reduce(
                    out=acc[:], in_=bass.AP(mx.tensor, mx[:].offset,
                                            [[mx[:].ap[0][0], 128], [1, 32], [32, 4]]),
                    axis=mybir.AxisListType.X, op=ALU.add)
                f2t = f2_all[:, t * 32:(t + 1) * 32]
                nc.vector.tensor_add(out=acc[:], in0=acc[:], in1=cen1[:, t * 32:(t + 1) * 32])
                nc.scalar.activation(out=f2t, in_=acc[:], func=AF.Relu)
                # adain stats
                sp = ps2.tile([1, 64], F32, tag="sp")
                nc.tensor.matmul(out=sp[:, 0:32], lhsT=ones128[:], rhs=f2t, start=True, stop=True)
                f2sq = sml.tile([128, 32], F32, tag="f2sq")
                nc.vector.tensor_mul(out=f2sq[:], in0=f2t, in1=f2t)
                nc.tensor.matmul(out=sp[:, 32:64], lhsT=ones128[:], rhs=f2sq[:], start=True, stop=True)
                nc.vector.tensor_add(out=s1acc[:], in0=s1acc[:], in1=sp[:, 0:32])
                nc.vector.tensor_add(out=s2acc[:], in0=s2acc[:], in1=sp[:, 32:64])

            # ---- adain finalize ----
            stat = keep.tile([1, 64], F32)
            nc.vector.tensor_scalar_mul(stat[:, 0:32], s1acc[:], 1.0 / V)
            m2 = keep.tile([1, 32], F32)
            nc.vector.tensor_mul(out=m2[:], in0=stat[:, 0:32], in1=s1acc[:])
            nc.vector.tensor_sub(out=m2[:], in0=s2acc[:], in1=m2[:])
            nc.vector.tensor_scalar_mul(m2[:], m2[:], 1.0 / (V - 1))
            nc.scalar.activation(out=m2[:], in_=m2[:], func=AF.Sqrt)
            nc.vector.tensor_scalar_add(m2[:], m2[:], 1e-8)
            nc.vector.reciprocal(out=stat[:, 32:64], in_=m2[:])
            ones1 = keep.tile([1, 128], F32)
            nc.vector.memset(ones1[:], 1.0)
            bc_ps = ps2.tile([128, 64], F32, tag="feat")
            nc.tensor.matmul(out=bc_ps[:], lhsT=ones1[:], rhs=stat[:], start=True, stop=True)
            bc = keep.tile([128, 64], F32)
            nc.scalar.copy(out=bc[:], in_=bc_ps[:])

            # ---- pass 1b: t = adain(f2), dc1 table ----
            for t in range(NT):
                tft = sml.tile([128, 10], F32, tag="tft")
                nc.sync.dma_start(out=tft[:], in_=tf[t * 128:(t + 1) * 128, :])
                hp = feat_matmul(t, tft[:], 10, lta, wbas[:], 64)
                f2t = f2_all[:, t * 32:(t + 1) * 32]
                xn = sml.tile([128, 32], F32, tag="xn")
                nc.vector.tensor_sub(out=xn[:], in0=f2t, in1=bc[:, 0:32])
                nc.vector.tensor_mul(out=xn[:], in0=xn[:], in1=bc[:, 32:64])
                g1 = sml.tile([128, 32], F32, tag="g1")
                nc.scalar.add(out=g1[:], in_=hp[:, 0:32], add=1.0)
                nc.vector.tensor_mul(out=xn[:], in0=xn[:], in1=g1[:])
                tt = t_all[:, t * 32:(t + 1) * 32]
                nc.vector.tensor_add(out=tt, in0=xn[:], in1=hp[:, 32:64])
                fp = feat_matmul(t, tt, 32, ltd1, wbd1s[:], 80)
                nc.scalar.copy(out=cend1[:, t * 16:(t + 1) * 16], in_=fp[:, 0:16])
                sup = sml.tile([128, 64], BF16, tag="sup2")
                nc.scalar.copy(out=sup[:], in_=fp[:, 16:80])
                nc.sync.dma_start(
                    out=bass.AP(t2[:].tensor, t * 128 * 128, [[128, 128], [1, 64]]),
                    in_=sup[:])

            # ================= pass 2: dc1 -> c1, dc2 table =================
            for t in range(NT):
                sg = mid.tile([128, NB * 64], BF16, tag="sg")
                gather(t, bass.AP(t2[:].tensor, 0, [[128, V], [1, 64]]), 64, sg)
                th = mid.tile([128, NB * 64], BF16, tag="th")
                theta_relu(t, bd2s, 64, th)
                nc.vector.tensor_mul(out=th[:], in0=th[:], in1=sg[:])
                mx = sml.tile([128, 64], F32, tag="mx128")
                nc.vector.tensor_reduce(
                    out=mx[:], in_=bass.AP(th.tensor, th[:].offset,
                                           [[th[:].ap[0][0], 128], [1, 64], [64, NB]]),
                    axis=mybir.AxisListType.X, op=ALU.max)
                acc = sml.tile([128, 16], F32, tag="acc16")
                nc.vector.tensor_reduce(
                    out=acc[:], in_=bass.AP(mx.tensor, mx[:].offset,
                                            [[mx[:].ap[0][0], 128], [1, 16], [16, 4]]),
                    axis=mybir.AxisListType.X, op=ALU.add)
                c1t = c1_all[:, t * 16:(t + 1) * 16]
                nc.vector.tensor_add(out=acc[:], in0=acc[:], in1=cend1[:, t * 16:(t + 1) * 16])
                nc.scalar.activation(out=c1t, in_=acc[:], func=AF.Relu)
                fp2 = feat_matmul(t, c1t, 16, ltd2, wbd2s[:], 15)
                nc.scalar.copy(out=cend2[:, t * 3:(t + 1) * 3], in_=fp2[:, 0:3])
                sup = sml.tile([128, 12], BF16, tag="sup3")
                nc.scalar.copy(out=sup[:], in_=fp2[:, 3:15])
                nc.sync.dma_start(
                    out=bass.AP(t3[:].tensor, t * 128 * 128, [[128, 128], [1, 12]]),
                    in_=sup[:])

            # ================= pass 3: dc2 -> sigmoid -> out =================
            for t in range(NT):
                sg = mid.tile([128, NB * 32], BF16, tag="sg")
                gather(t, bass.AP(t3[:].tensor, 0, [[128, V], [1, 32]]), 32, sg)
                th = mid.tile([128, NB * 12], BF16, tag="th")
                theta_relu(t, bd3s, 12, th)
                sgv = bass.AP(sg.tensor, sg[:].offset,
                              [[sg[:].ap[0][0], 128], [32, NB], [1, 12]])
                nc.vector.tensor_tensor(out=th[:], in0=th[:], in1=sgv, op=ALU.mult)
                mx = sml.tile([128, 12], F32, tag="mx12")
                nc.vector.tensor_reduce(
                    out=mx[:], in_=bass.AP(th.tensor, th[:].offset,
                                           [[th[:].ap[0][0], 128], [1, 12], [12, NB]]),
                    axis=mybir.AxisListType.X, op=ALU.max)
                acc = sml.tile([128, 3], F32, tag="acc3")
                nc.vector.tensor_reduce(
                    out=acc[:], in_=bass.AP(mx.tensor, mx[:].offset,
                                            [[mx[:].ap[0][0], 128], [1, 3], [3, 4]]),
                    axis=mybir.AxisListType.X, op=ALU.add)
                nc.vector.tensor_add(out=acc[:], in0=acc[:], in1=cend2[:, t * 3:(t + 1) * 3])
                sig = sml.tile([128, 3], F32, tag="sig")
                nc.scalar.activation(out=sig[:], in_=acc[:], func=AF.Sigmoid)
                nc.sync.dma_start(out=out[t * 128:(t + 1) * 128, :], in_=sig[:])

    _split_excess_waits(nc)
    _encode_reloads(nc)
    return nc


_NC_CACHE = None


def _host_consts(inputs):
    bf = ml_dtypes.bfloat16
    repsel = np.zeros((16, 128), np.float32)
    for p in range(128):
        repsel[p % 16, p] = 1.0
    return {
        'bd0': _block_dirs(_normalize_cols(np.asarray(inputs['conv0_dirs'])), 64).astype(bf),
        'bd1': _block_dirs(_normalize_cols(np.asarray(inputs['conv1_dirs'])), 128).astype(bf),
        'bd2': _block_dirs(_normalize_cols(np.asarray(inputs['dc1_dirs'])), 64).astype(bf),
        'bd3': _block_dirs(_normalize_cols(np.asarray(inputs['dc2_dirs'])), 12).astype(bf),
        'wb1': np.vstack([np.asarray(inputs['conv1_w']), np.asarray(inputs['conv1_b'])[None]]).astype(bf),
        'wba': np.vstack([np.asarray(inputs['adain_w']), np.asarray(inputs['adain_b'])[None]]).astype(bf),
        'wbd1': np.vstack([np.asarray(inputs['dc1_w']), np.asarray(inputs['dc1_b'])[None]]).astype(bf),
        'wbd2': np.vstack([np.asarray(inputs['dc2_w']), np.asarray(inputs['dc2_b'])[None]]).astype(bf),
        'repsel': repsel,
        'identin': np.eye(128, dtype=np.float32),
        'iota32': np.tile(np.arange(V, dtype=np.int32)[None, :], (128, 1)),
    }


def _dist_operands(x):
    bf = ml_dtypes.bfloat16
    f = np.float32
    sq = (x * x).sum(1)
    xh = x.astype(bf); xl = (x - xh.astype(f)).astype(bf)
    sqh = sq.astype(bf); sql = (sq - sqh.astype(f)).astype(bf)
    c = -sq; ch = c.astype(bf); cl = (c - ch.astype(f)).astype(bf)
    ones = np.ones(V, bf)
    x2h = (2.0 * xh.astype(f)).astype(bf)
    x2l = (2.0 * xl.astype(f)).astype(bf)
    lhsT = np.stack([*xh.T, *xl.T, *xh.T, sqh, sql, ones, ones])
    rhs = np.stack([*x2h.T, *x2h.T, *x2l.T, -ones, -ones, ch, cl])
    return lhsT.astype(bf), rhs.astype(bf)


def kernel(**inputs):
    global _NC_CACHE
    from concourse.bass_utils import run_bass_kernel_spmd

    src = np.ascontiguousarray(np.asarray(inputs['source'], dtype=np.float32))
    tf = np.ascontiguousarray(np.asarray(inputs['target_feature'], dtype=np.float32))
    consts = {k: np.ascontiguousarray(v) for k, v in _host_consts(inputs).items()}
    if _NC_CACHE is None:
        _NC_CACHE = build_kernel()
    nc = _NC_CACHE
    in_maps = []
    for b in range(B):
        l13, r13 = _dist_operands(src[b])
        in_maps.append(dict(consts, source=src[b], target_feature=tf[b],
                            lhs13i=np.ascontiguousarray(l13),
                            rhs13i=np.ascontiguousarray(r13)))
    res = run_bass_kernel_spmd(nc, in_maps, list(range(B)))
    return np.stack([res.results[b]['out'] for b in range(B)]).astype(np.float32)


if __name__ == '__main__':
    inp = dict(np.load('/root/problem/dev/inputs.npz'))
    o = kernel(**inp)
    print(o.shape, o.dtype)
